# revision 20
# baseline (speedup 1.0000x reference)
"""Self-contained Trainium2 Bass kernel for 4-layer GraphSAGE (nn_LASAGE).

Strategy (v8 - packed 1920-idx gather calls, single_packet=False):
  - Nodes dst-sharded across 8 cores (6250/core, padded to 6272 = 49 blocks of 128).
  - Aggregation is done POST-matmul: agg(x)@Wl == agg(x@Wl), so per layer each
    core computes y = h @ Wl for its own shard; the full Y table [50176, d] is
    replicated via THREE chunked AllGathers (blocks [0:22) [22:39) [39:49)),
    kicked as soon as each chunk's rows are written. Edges gather y[src] rows
    with dma_gather (fp8e4, 256B rows) on 4 SWDGE queues.
  - Gather calls carry up to 15 tiles (1920 idxs) each: single_packet=False
    avoids the 16KB SDMA packet-coalescing limit that capped calls at 1024
    idxs (and wedges the device beyond it). Calls PACK ACROSS dst-block
    boundaries within a chunk's edge stream (tiles may straddle blocks; a
    straddling tile gets one one-hot matmul per block it spans), cutting the
    GpSimd descriptor-gen ucode cost (~994ns fixed + ~2.3ns/idx, the kernel's
    main bottleneck) from ~600 calls to ~171.
  - Per (chunk, block) the edge segment is padded to the max count over cores
    (SPMD: one program, per-core idx/dstl images) with idx=0 / dstcol=-1.
  - Scatter-add into dst blocks via one-hot matmuls on the PE. One-hots are
    {0,1} BF16, built per (block, chunk) with a single DVE IS_EQ against a
    host-provided bf16 dst-column map; matmuls mix fp8 gather tiles with bf16
    one-hots (legal: only fp32 operands must match).
  - Mean-normalization moves to the edges:
      out = invd[dst] * (gather_sum + degc[dst]*(x@Wr) + degc[dst]*b)
    with degc = max(deg,1) pre-scaled Wr inputs and an invd epilogue.
  - All dense operands (x, weights, h storage) are bf16; psum stays fp32.
  - Layer1 fuses conv0+conv1 (concat -> 256 feat). Layer3 (output, d=64) uses
    non-transposed psum (lhsT=onehot) so rows DMA straight to the output;
    its fp8 table rows are 256-wide with only cols 0:64 valid.
  - Rejected on measurement: 2560+/3840-idx calls (per-idx ucode cost rises,
    drain-paced), fp8 DoubleRow scatter matmuls (PE is column-throughput
    bound, no win), 4-chunk AG layout, cross-layer gather preflight (stalls
    on the next table's AllGather mid-loop), indirect_dma_start (HW path
    returns garbage on this runtime).
"""
import sys, os, types

sys.path.insert(0, "/opt/trn_rl_repo")
import numpy as np

N = 50000
E = 800000
NCORES = 8
S = N // NCORES            # 6250 real nodes per core
SP = 6272                  # padded (49 blocks of 128)
NBLK = SP // 128
D1 = 256                   # concat(h0, h1)
DM = 256
DO = 64
MAXI = int(os.environ.get("K_MAXI", "1920"))   # max idxs per dma_gather call
TPC = MAXI // 128          # tiles per full call
CPC = MAXI // 16           # idx-image cols per call
SINGLE_PACKET = os.environ.get("K_SP", "0") == "1"   # >1024 idxs needs False
DOUBLE_ROW = os.environ.get("K_DR", "0") == "1"
NCH = 3
CBLK = [0, 22, 39, 49]       # chunk boundaries in blocks (small tail AG)
CST = [b * 128 for b in CBLK[:-1]]              # chunk start rows (per core)
CSZ = [(CBLK[i + 1] - CBLK[i]) * 128 for i in range(NCH)]   # [2816, 2176, 1280]
TBL = [NCORES * s for s in CSZ]                 # AG table rows (int16-safe)
DMA_SCRATCH = int(os.environ.get("K_SCRATCH", "16384"))  # SWDGE ring carveout


def _install_hooks():
    """antenv.axon_hooks shim so trace=True works in this image (optional)."""
    try:
        import antenv
        if "antenv.axon_hooks" not in sys.modules:
            mod = types.ModuleType("antenv.axon_hooks")
            mod._hook = None
            mod.set_axon_ntff_profile_hook = lambda h: setattr(mod, "_hook", h)
            mod.get_axon_ntff_profile_hook = lambda: mod._hook
            sys.modules["antenv.axon_hooks"] = mod
            antenv.axon_hooks = mod
        from antenv.axon_hooks import get_axon_ntff_profile_hook, set_axon_ntff_profile_hook
        if get_axon_ntff_profile_hook() is None:
            from trn_agent_boot.trn_boot import _ntff_profile_via_ctypes
            set_axon_ntff_profile_hook(_ntff_profile_via_ctypes("/opt/axon/libaxon_pjrt.so"))
        import concourse.bass_utils as bu
        bu.upload_artifacts = lambda tmpdir: f"file://{tmpdir}"
    except Exception:
        pass


def _preprocess(edge_index):
    """Per-core edge streams grouped by (chunk, dst block), max-padded per
    (chunk, block) so the tile/call structure is identical across cores."""
    src = np.asarray(edge_index[0], np.int64)
    dst = np.asarray(edge_index[1], np.int64)
    core = dst // S
    dl = (dst % S).astype(np.int64)
    blk = dl // 128
    col = dl % 128
    sloc = src % S
    chunk = np.digitize(sloc, CST[1:])
    cst = np.asarray(CST)[chunk]
    csz = np.asarray(CSZ)[chunk]
    grow = (src // S) * csz + (sloc - cst)   # row within its chunk-table

    deg = np.bincount(core * S + dl, minlength=N).reshape(NCORES, S)

    order = np.lexsort((grow, blk, chunk, core))
    core_s, ch_s, blk_s, col_s, row_s = (core[order], chunk[order], blk[order],
                                         col[order], grow[order])

    key = (core_s * NCH + ch_s) * NBLK + blk_s
    counts = np.bincount(key, minlength=NCORES * NCH * NBLK).reshape(NCORES, NCH, NBLK)
    cap_hb = counts.max(axis=0).astype(np.int64)     # [NCH, NBLK] segment capacity

    seg_off = np.zeros((NCH, NBLK), np.int64)
    seg_off[:, 1:] = np.cumsum(cap_hb, axis=1)[:, :-1]
    stream_len = cap_hb.sum(axis=1)                  # [NCH]
    tiles_h = np.ceil(stream_len / 128).astype(np.int64)
    pad_len = tiles_h * 128

    srcpad = np.zeros((NCORES, NCH), dtype=object)
    colpad = np.zeros((NCORES, NCH), dtype=object)
    for c in range(NCORES):
        for h in range(NCH):
            srcpad[c, h] = np.zeros(int(pad_len[h]), np.int64)
            colpad[c, h] = np.full(int(pad_len[h]), -1, np.int64)
    grp = key
    first = np.r_[True, grp[1:] != grp[:-1]]
    gidx = np.arange(len(grp)) - np.maximum.accumulate(np.where(first, np.arange(len(grp)), 0))
    pos = seg_off[ch_s, blk_s] + gidx
    for c in range(NCORES):
        m = core_s == c
        for h in range(NCH):
            mh = m & (ch_s == h)
            p = pos[mh]
            srcpad[c, h][p] = row_s[mh]
            colpad[c, h][p] = col_s[mh]

    return {
        "cap_hb": cap_hb, "seg_off": seg_off, "tiles_h": tiles_h,
        "srcpad": srcpad, "colpad": colpad, "deg": deg,
    }


def _build_callplan(pre):
    """Compile-time plan shared by every core.

    calls[ci] = dict(h, k, tile_base, b_first)  — up to TPC tiles of chunk h.
    pairs: for each (block b, chunk h): list of (ci, slot, paircol) where
      paircol indexes the dstl image column for this (tile, block) one-hot.
    dstl columns are ordered by (h, b, tile) so each (b, h)'s columns are
      contiguous: oh_rng[b][h] = (p0, np).
    """
    cap_hb, seg_off, tiles_h = pre["cap_hb"], pre["seg_off"], pre["tiles_h"]
    calls = []
    tile_call = {}           # (h, t) -> (ci, slot)
    for h in range(NCH):
        nt = int(tiles_h[h])
        done = 0
        while done < nt:
            k = min(TPC, nt - done)
            ci = len(calls)
            calls.append(dict(h=h, k=k, tile_base=done, b_first=None))
            for j in range(k):
                tile_call[(h, done + j)] = (ci, j)
            done += k

    # (tile, block) intersections per chunk, ordered by (h, b, t)
    pair_list = []           # global: (h, b, t, ci, slot)
    block_pairs = {b: {h: [] for h in range(NCH)} for b in range(NBLK)}
    oh_rng = {b: {} for b in range(NBLK)}
    for h in range(NCH):
        for b in range(NBLK):
            s0 = int(seg_off[h, b])
            s1 = s0 + int(cap_hb[h, b])
            if s1 == s0:
                oh_rng[b][h] = (len(pair_list), 0)
                continue
            t0, t1 = s0 // 128, (s1 - 1) // 128
            p0 = len(pair_list)
            for t in range(t0, t1 + 1):
                ci, slot = tile_call[(h, t)]
                paircol = len(pair_list)
                pair_list.append((h, b, t, ci, slot))
                block_pairs[b][h].append((ci, slot, paircol))
                if calls[ci]["b_first"] is None:
                    calls[ci]["b_first"] = b
            oh_rng[b][h] = (p0, len(pair_list) - p0)
    for cl in calls:
        if cl["b_first"] is None:       # tail-pad-only call
            cl["b_first"] = NBLK - 1
    # round-robin queues
    for i, cl in enumerate(calls):
        cl["q"] = i % 4
    mth = [max((oh_rng[b][h][1] for b in range(NBLK)), default=1) or 1
           for h in range(NCH)]
    return calls, block_pairs, oh_rng, pair_list, mth


def _idx_arrays(pre, calls, pair_list, core):
    """int16 idx image [128, ncalls*CPC] and per-pair dst-col map (bf16)."""
    import ml_dtypes as _ml
    ncalls = len(calls)
    idx_img = np.zeros((16, ncalls * CPC), np.int16)
    npairs = len(pair_list)
    dstloc = np.full((128, npairs), -1, np.int64)
    cap_hb, seg_off = pre["cap_hb"], pre["seg_off"]
    counts = None
    for ci, cl in enumerate(calls):
        h, k, tb = cl["h"], cl["k"], cl["tile_base"]
        e0 = tb * 128
        nidx = k * 128
        seg_src = pre["srcpad"][core, h][e0:e0 + nidx]
        idx_img[:, ci * CPC: ci * CPC + (nidx // 16)] = seg_src.reshape(-1, 16).T.astype(np.int16)
    # per-core column maps: valid only inside this core's real count range
    # (colpad already holds -1 at padded positions)
    for paircol, (h, b, t, ci, slot) in enumerate(pair_list):
        seg_col = pre["colpad"][core, h][t * 128:(t + 1) * 128]
        s0 = int(seg_off[h, b])
        s1 = s0 + int(cap_hb[h, b])
        j = np.arange(t * 128, (t + 1) * 128)
        inblk = (j >= s0) & (j < s1)
        dstloc[:, paircol] = np.where(inblk, seg_col, -1)
    return np.tile(idx_img, (8, 1)), dstloc.astype(np.float32).astype(_ml.bfloat16)


def _build_bass(pre, calls, block_pairs, oh_rng, npairs, mth):
    import concourse.bass as bass
    import concourse.bacc as bacc
    import concourse.mybir as mybir
    import concourse.tile as tile

    FP32 = mybir.dt.float32
    BF16 = mybir.dt.bfloat16
    F8 = mybir.dt.float8e4
    I16 = mybir.dt.int16
    AL = mybir.AluOpType
    AF = mybir.ActivationFunctionType

    ncalls = len(calls)
    MT = max(mth)            # iota needs to cover the largest oh tile

    nc = bacc.Bacc("TRN2", target_bir_lowering=False, debug=False,
                   enable_asserts=False, num_devices=NCORES, num_swdge_queues=4,
                   dynamic_dma_scratch_size=DMA_SCRATCH)

    x0T = nc.dram_tensor("x0T", [128, SP], BF16, kind="ExternalInput")
    x1T = nc.dram_tensor("x1T", [128, SP], BF16, kind="ExternalInput")
    x0dT = nc.dram_tensor("x0dT", [128, SP], BF16, kind="ExternalInput")
    x1dT = nc.dram_tensor("x1dT", [128, SP], BF16, kind="ExternalInput")
    wl0 = nc.dram_tensor("wl0", [128, 128], BF16, kind="ExternalInput")
    wr0 = nc.dram_tensor("wr0", [128, 128], BF16, kind="ExternalInput")
    wl1 = nc.dram_tensor("wl1", [128, 128], BF16, kind="ExternalInput")
    wr1 = nc.dram_tensor("wr1", [128, 128], BF16, kind="ExternalInput")
    wlm = nc.dram_tensor("wlm", [256, 256], BF16, kind="ExternalInput")
    wrm = nc.dram_tensor("wrm", [256, 256], BF16, kind="ExternalInput")
    wlo = nc.dram_tensor("wlo", [256, 64], BF16, kind="ExternalInput")
    wro = nc.dram_tensor("wro", [256, 64], BF16, kind="ExternalInput")
    b01d = nc.dram_tensor("b01", [1, 256], BF16, kind="ExternalInput")
    bmd = nc.dram_tensor("bm", [1, 256], BF16, kind="ExternalInput")
    bod = nc.dram_tensor("bo", [1, 64], BF16, kind="ExternalInput")
    idxd = nc.dram_tensor("idx", [128, ncalls * CPC], I16, kind="ExternalInput")
    dstld = nc.dram_tensor("dstl", [128, npairs], BF16, kind="ExternalInput")
    invrd = nc.dram_tensor("invr", [128, SP], BF16, kind="ExternalInput")
    degrd = nc.dram_tensor("degr", [128, SP], BF16, kind="ExternalInput")
    invcd = nc.dram_tensor("invc", [128, NBLK], FP32, kind="ExternalInput")
    outd = nc.dram_tensor("out", [S, DO], FP32, kind="ExternalOutput")

    with tile.TileContext(nc) as tc:
        with (
            tc.tile_pool(name="const", bufs=1) as cp,
            tc.tile_pool(name="acts", bufs=1) as hp,
            tc.tile_pool(name="g", bufs=15) as gp,
            tc.tile_pool(name="oh", bufs=2) as ohp,
            tc.tile_pool(name="xs", bufs=6) as xsp,
            # PSUM budget (8 banks): ps0/ps1 (3 bufs each = 6 banks) + py (2)
            tc.tile_pool(name="ps", bufs=3, space="PSUM") as psp,
            tc.tile_pool(name="psy", bufs=2, space="PSUM") as psyp,
            tc.tile_pool(name="ev", bufs=6) as evp,
            tc.tile_pool(name="dram", bufs=1, space="DRAM") as dp,
        ):
            def load(name, dt_, shape, src):
                t = cp.tile(shape, dt_, name=name)
                nc.sync.dma_start(out=t[:], in_=src)
                return t

            wl0t = load("wl0t", BF16, [128, 128], wl0[:])
            wr0t = load("wr0t", BF16, [128, 128], wr0[:])
            wl1t = load("wl1t", BF16, [128, 128], wl1[:])
            wr1t = load("wr1t", BF16, [128, 128], wr1[:])
            wlmt = [load(f"wlmt{i}", BF16, [128, 256], wlm[i * 128:(i + 1) * 128, :]) for i in range(2)]
            wrmt = [load(f"wrmt{i}", BF16, [128, 256], wrm[i * 128:(i + 1) * 128, :]) for i in range(2)]
            wlot = [load(f"wlot{i}", BF16, [128, 64], wlo[i * 128:(i + 1) * 128, :]) for i in range(2)]
            wrot = [load(f"wrot{i}", BF16, [128, 64], wro[i * 128:(i + 1) * 128, :]) for i in range(2)]
            b01t = load("b01t", BF16, [1, 256], b01d[:])
            bmt = load("bmt", BF16, [1, 256], bmd[:])
            bot = load("bot", BF16, [1, 64], bod[:])
            idxt = load("idxt", I16, [128, ncalls * CPC], idxd[:])
            invr = load("invrt", BF16, [128, SP], invrd[:])
            degr = load("degrt", BF16, [128, SP], degrd[:])
            invc = load("invct", FP32, [128, NBLK], invcd[:])
            dstl = load("dstlt", BF16, [128, npairs], dstld[:])

            iota_i = cp.tile([128, MT, 128], mybir.dt.int32, name="iota_i")
            nc.gpsimd.iota(iota_i[:], pattern=[[0, MT], [1, 128]], base=0,
                           channel_multiplier=0)
            iota_bf = cp.tile([128, MT, 128], BF16, name="iota_bf")
            nc.vector.tensor_copy(out=iota_bf[:], in_=iota_i[:])

            warm_own = dp.tile([8, 256], F8, name="warm_own")
            warm_tab = dp.tile([64, 256], F8, name="warm_tab",
                               addr_space="Shared" if NCORES > 4 else "Local")
            wz = evp.tile([8, 256], F8, name="wz", tag="wz")
            nc.vector.memset(wz[:], 0.0)
            nc.sync.dma_start(out=warm_own[:], in_=wz[:])
            nc.gpsimd.collective_compute(
                "AllGather", AL.bypass, replica_groups=[list(range(NCORES))],
                ins=[warm_own[:]], outs=[warm_tab[:]])

            hT = [hp.tile([128, SP], BF16, name=f"hT{i}") for i in range(2)]
            h2T = [hp.tile([128, SP], BF16, name=f"h2T{i}") for i in range(2)]

            shared = "Shared" if NCORES > 4 else "Local"

            def mk_tables(name, width):
                own = [dp.tile([CSZ[h], width], F8, name=f"{name}_own{h}")
                       for h in range(NCH)]
                tab = [dp.tile([TBL[h], width], F8, name=f"{name}{h}",
                               addr_space=shared) for h in range(NCH)]
                return own, tab

            y01_own, Y01 = mk_tables("y01", D1)
            ym_own, Ym = mk_tables("ym", DM)
            yo_own, Yo = mk_tables("yo", 256)

            def chunk_of_block(b):
                for h in range(NCH):
                    if b < CBLK[h + 1]:
                        return h
                raise ValueError(b)

            def write_y(dsts, b, src_tile, dcols):
                h = chunk_of_block(b)
                r0 = b * 128 - CST[h]
                nc.sync.dma_start(out=dsts[h][r0:r0 + 128, 0:dcols],
                                  in_=src_tile[:, 0:dcols])

            RG = [list(range(NCORES))]

            def blk_sl(b):
                return slice(b * 128, (b + 1) * 128)

            def make_ags(own, tab):
                def mk(h):
                    def f():
                        nc.gpsimd.collective_compute(
                            "AllGather", AL.bypass, replica_groups=RG,
                            ins=[own[h][:]], outs=[tab[h][:]])
                    return f
                return [mk(h) for h in range(NCH)]

            def load_oh(b):
                """Build this block's one-hot tiles with a single DVE IS_EQ
                per chunk (bf16 out -> 2x DVE mode)."""
                tiles = {}
                for h in range(NCH):
                    start, nt = oh_rng[b][h]
                    if nt == 0:
                        tiles[h] = (None, start)
                        continue
                    t = ohp.tile([128, mth[h], 128], BF16, name=f"ohb{h}",
                                 tag=f"oh{h}")
                    nc.vector.tensor_tensor(
                        out=t[:, 0:nt, :], in0=iota_bf[:, 0:nt, :],
                        in1=dstl[:, start:start + nt].to_broadcast([128, nt, 128]),
                        op=AL.is_equal)
                    tiles[h] = (t, start)
                return tiles

            # AG kick: chunk i kicks a few blocks after its rows are written
            # so the kick's input-wait is already satisfied; tail at loop end.
            ag_at = {CBLK[1] + 1: [0], CBLK[2] + 1: [1], CBLK[3] - 1: [2]}

            _pg_cache = {}

            def pair_groups(b):
                """Group block b's (tile, block) one-hot pairs for DoubleRow:
                two consecutive entries from the same call with adjacent slots
                and adjacent one-hot columns form one fp8 DoubleRow matmul."""
                if b in _pg_cache:
                    return _pg_cache[b]
                flat = [(h, p) for h in range(NCH) for p in block_pairs[b][h]]
                groups = []
                i = 0
                while i < len(flat):
                    if (DOUBLE_ROW and i + 1 < len(flat)
                            and flat[i][0] == flat[i + 1][0]
                            and flat[i][1][0] == flat[i + 1][1][0]
                            and flat[i][1][1] + 1 == flat[i + 1][1][1]
                            and flat[i][1][2] + 1 == flat[i + 1][1][2]):
                        groups.append([flat[i], flat[i + 1]])
                        i += 2
                    else:
                        groups.append([flat[i]])
                        i += 1
                out = [(g, gi == len(groups) - 1) for gi, g in enumerate(groups)]
                _pg_cache[b] = out
                return out

            # gather emission: per-chunk pointers; chunk 0/1 get a deep
            # lookahead window, chunk 2 a shallow one (its AG lands during
            # the layer's first blocks).
            WCH = [10, 7, 2]
            WPF = [0, None, None]       # cross-layer preflight windows
            PF_B = 99                   # preflight disabled (regressed on HW)
            calls_by_h = {h: [ci for ci, cl in enumerate(calls) if cl["h"] == h]
                          for h in range(NCH)}

            def new_gst():
                return {"ptr": {h: 0 for h in range(NCH)}, "g": {}}

            def emit_for(gst, Ytab, b, wch):
                for h in range(NCH):
                    w = wch[h]
                    if w is None:
                        continue
                    lst = calls_by_h[h]
                    while (gst["ptr"][h] < len(lst)
                           and calls[lst[gst["ptr"][h]]]["b_first"] <= b + w):
                        ci = lst[gst["ptr"][h]]
                        cl = calls[ci]
                        k = cl["k"]
                        g = gp.tile([128, TPC, 256], F8, name="g", tag="g")
                        nc.gpsimd.dma_gather(
                            out_ap=g[:, 0:k, :],
                            in_ap=Ytab[h][:],
                            idxs_ap=idxt[:, ci * CPC: ci * CPC + (k * 128) // 16],
                            num_idxs=k * 128, num_idxs_reg=k * 128,
                            elem_size=256, queue_num=cl["q"], single_packet=SINGLE_PACKET)
                        gst["g"][ci] = g
                        gst["ptr"][h] += 1

            gst1, gst2, gst3 = new_gst(), new_gst(), new_gst()

            # ================= L1 pre: y01_own = [x0@Wl0 | x1@Wl1] =========
            ags01 = make_ags(y01_own, Y01)
            ag_at_pre = {CBLK[1] - 1: 0, CBLK[2] - 1: 1, CBLK[3] - 1: 2}
            for b in range(NBLK):
                x0b = xsp.tile([128, 128], BF16, name="x0b", tag="x0b")
                nc.sync.dma_start(out=x0b[:], in_=x0T[:, blk_sl(b)])
                x1b = xsp.tile([128, 128], BF16, name="x1b", tag="x1b")
                nc.sync.dma_start(out=x1b[:], in_=x1T[:, blk_sl(b)])
                py0 = psp.tile([128, 128], FP32, name="py0", tag="ps0")
                py1 = psp.tile([128, 128], FP32, name="py1", tag="ps1")
                nc.tensor.matmul(py0[:], lhsT=x0b[:], rhs=wl0t[:], start=True, stop=True)
                nc.tensor.matmul(py1[:], lhsT=x1b[:], rhs=wl1t[:], start=True, stop=True)
                evy = evp.tile([128, 256], F8, name="evy", tag="evy", padded_shape=[128, 512])
                nc.vector.tensor_copy(out=evy[:, 0:128], in_=py0[:])
                nc.vector.tensor_copy(out=evy[:, 128:256], in_=py1[:])
                write_y(y01_own, b, evy, D1)
                if b in ag_at_pre:
                    ags01[ag_at_pre[b]]()
                if b >= PF_B:
                    emit_for(gst1, Y01, b - PF_B, WPF)

            # ================= aggregation layer (L1/L2) =====================
            def agg_layer(Ytab, wr_tiles, bias_t, h_src, h_dst, wl_next, y_next,
                          d_next, ags_next, gst, pf_gst=None, pf_tab=None):
                gtiles = gst["g"]

                for b in range(NBLK):
                    emit_for(gst, Ytab, b, WCH)
                    ohb = load_oh(b)
                    ps0 = psp.tile([128, 128], FP32, name="ps0", tag="ps0")
                    ps1 = psp.tile([128, 128], FP32, name="ps1", tag="ps1")
                    if h_src is None:
                        x0b = xsp.tile([128, 128], BF16, name="x0b2", tag="xd0")
                        nc.sync.dma_start(out=x0b[:], in_=x0dT[:, blk_sl(b)])
                        x1b = xsp.tile([128, 128], BF16, name="x1b2", tag="xd1")
                        nc.sync.dma_start(out=x1b[:], in_=x1dT[:, blk_sl(b)])
                        nc.tensor.matmul(ps0[:], lhsT=wr0t[:], rhs=x0b[:], start=True, stop=False)
                        nc.tensor.matmul(ps1[:], lhsT=wr1t[:], rhs=x1b[:], start=True, stop=False)
                    else:
                        hd0 = evp.tile([128, 128], BF16, name="hd0", tag="hd0")
                        nc.vector.tensor_tensor(out=hd0[:], in0=h_src[0][:, blk_sl(b)],
                                                in1=degr[:, blk_sl(b)], op=AL.mult)
                        hd1 = evp.tile([128, 128], BF16, name="hd1", tag="hd1")
                        nc.vector.tensor_tensor(out=hd1[:], in0=h_src[1][:, blk_sl(b)],
                                                in1=degr[:, blk_sl(b)], op=AL.mult)
                        nc.tensor.matmul(ps0[:], lhsT=wr_tiles[0][:, 0:128], rhs=hd0[:], start=True, stop=False)
                        nc.tensor.matmul(ps0[:], lhsT=wr_tiles[1][:, 0:128], rhs=hd1[:], start=False, stop=False)
                        nc.tensor.matmul(ps1[:], lhsT=wr_tiles[0][:, 128:256], rhs=hd0[:], start=True, stop=False)
                        nc.tensor.matmul(ps1[:], lhsT=wr_tiles[1][:, 128:256], rhs=hd1[:], start=False, stop=False)
                    nc.tensor.matmul(ps0[:], lhsT=bias_t[0:1, 0:128], rhs=degr[0:1, blk_sl(b)],
                                     start=False, stop=False)
                    nc.tensor.matmul(ps1[:], lhsT=bias_t[0:1, 128:256], rhs=degr[0:1, blk_sl(b)],
                                     start=False, stop=False)
                    for grp, last in pair_groups(b):
                        h, (ci, slot, paircol) = grp[0]
                        g = gtiles[ci]
                        oht, start = ohb[h]
                        j = paircol - start
                        if len(grp) == 2:
                            nc.tensor.matmul(ps0[:], lhsT=g[:, slot:slot + 2, 0:128],
                                             rhs=oht[:, j:j + 2, :], start=False, stop=last,
                                             perf_mode=mybir.MatmulPerfMode.DoubleRow)
                            nc.tensor.matmul(ps1[:], lhsT=g[:, slot:slot + 2, 128:256],
                                             rhs=oht[:, j:j + 2, :], start=False, stop=last,
                                             perf_mode=mybir.MatmulPerfMode.DoubleRow)
                        else:
                            nc.tensor.matmul(ps0[:], lhsT=g[:, slot, 0:128], rhs=oht[:, j, :],
                                             start=False, stop=last)
                            nc.tensor.matmul(ps1[:], lhsT=g[:, slot, 128:256], rhs=oht[:, j, :],
                                             start=False, stop=last)
                    # epilogue: h = relu(ps) * invd  (relu commutes with the
                    # positive per-column scale)
                    rt0 = evp.tile([128, 128], BF16, name="rt0", tag="rt0")
                    nc.scalar.activation(rt0[:], ps0[:], AF.Relu)
                    nc.vector.tensor_tensor(out=h_dst[0][:, blk_sl(b)], in0=rt0[:],
                                            in1=invr[:, blk_sl(b)], op=AL.mult)
                    rt1 = evp.tile([128, 128], BF16, name="rt1", tag="rt1")
                    nc.scalar.activation(rt1[:], ps1[:], AF.Relu)
                    nc.vector.tensor_tensor(out=h_dst[1][:, blk_sl(b)], in0=rt1[:],
                                            in1=invr[:, blk_sl(b)], op=AL.mult)
                    pyn = psyp.tile([128, d_next], FP32, name="pyn", tag="py",
                                    padded_shape=[128, 256])
                    nc.tensor.matmul(pyn[:], lhsT=h_dst[0][:, blk_sl(b)], rhs=wl_next[0][:],
                                     start=True, stop=False)
                    nc.tensor.matmul(pyn[:], lhsT=h_dst[1][:, blk_sl(b)], rhs=wl_next[1][:],
                                     start=False, stop=True)
                    evn = evp.tile([128, d_next], F8, name="evn", tag="evy",
                                   padded_shape=[128, 512])
                    nc.vector.tensor_copy(out=evn[:], in_=pyn[:])
                    write_y(y_next, b, evn, d_next)
                    if b in ag_at:
                        for hh in ag_at[b]:
                            ags_next[hh]()
                    if pf_gst is not None and b >= PF_B:
                        emit_for(pf_gst, pf_tab, b - PF_B, WPF)

            agg_layer(Y01, None, b01t, None, hT, wlmt, ym_own, DM,
                      make_ags(ym_own, Ym), gst1, gst2, Ym)
            agg_layer(Ym, wrmt, bmt, hT, h2T, wlot, yo_own, DO,
                      make_ags(yo_own, Yo), gst2, gst3, Yo)

            # ================= L3: out[node, 64] ============================
            gtiles3 = gst3["g"]

            for b in range(NBLK):
                emit_for(gst3, Yo, b, WCH)
                ohb = load_oh(b)
                ps3 = psp.tile([128, DO], FP32, name="ps3", tag="ps0",
                               padded_shape=[128, 128])
                hd0 = evp.tile([128, 128], BF16, name="hd20", tag="hd0")
                nc.vector.tensor_tensor(out=hd0[:], in0=h2T[0][:, blk_sl(b)],
                                        in1=degr[:, blk_sl(b)], op=AL.mult)
                hd1 = evp.tile([128, 128], BF16, name="hd21", tag="hd1")
                nc.vector.tensor_tensor(out=hd1[:], in0=h2T[1][:, blk_sl(b)],
                                        in1=degr[:, blk_sl(b)], op=AL.mult)
                nc.tensor.matmul(ps3[:], lhsT=hd0[:], rhs=wrot[0][:],
                                 start=True, stop=False)
                nc.tensor.matmul(ps3[:], lhsT=hd1[:], rhs=wrot[1][:],
                                 start=False, stop=False)
                nc.tensor.matmul(ps3[:], lhsT=degr[0:1, blk_sl(b)], rhs=bot[0:1, :],
                                 start=False, stop=False)
                for grp, last in pair_groups(b):
                    h, (ci, slot, paircol) = grp[0]
                    g3 = gtiles3[ci]
                    oht, start = ohb[h]
                    j = paircol - start
                    if len(grp) == 2:
                        nc.tensor.matmul(ps3[:], lhsT=oht[:, j:j + 2, :],
                                         rhs=g3[:, slot:slot + 2, 0:64],
                                         start=False, stop=last,
                                         perf_mode=mybir.MatmulPerfMode.DoubleRow)
                    else:
                        nc.tensor.matmul(ps3[:], lhsT=oht[:, j, :], rhs=g3[:, slot, 0:64],
                                         start=False, stop=last)
                osb = evp.tile([128, DO], FP32, name="osb", tag="osb")
                nc.scalar.activation(osb[:], ps3[:], AF.Copy,
                                     scale=invc[:, b:b + 1])
                rows = min(128, S - b * 128)
                nc.sync.dma_start(out=outd[b * 128: b * 128 + rows, :],
                                  in_=osb[0:rows, :])

    nc.finalize()
    return nc


_CACHE = {}


def _make_inmaps(inputs, pre, calls, pair_list):
    import ml_dtypes as _ml
    BF = _ml.bfloat16
    x0 = np.asarray(inputs["x0"], np.float32)
    x1 = np.asarray(inputs["x1"], np.float32)
    deg = pre["deg"]
    bf16 = lambda a: np.ascontiguousarray(a).astype(BF)
    in_maps = []
    for c in range(NCORES):
        degc = np.maximum(deg[c], 1.0).astype(np.float32)
        invd = (1.0 / degc).astype(np.float32)
        degc_p = np.ones(SP, np.float32)
        degc_p[:S] = degc
        invd_p = np.ones(SP, np.float32)
        invd_p[:S] = invd
        idx_img, dstloc = _idx_arrays(pre, calls, pair_list, c)
        x0c = np.zeros((128, SP), np.float32)
        x0c[:, :S] = x0[c * S:(c + 1) * S, :].T
        x1c = np.zeros((128, SP), np.float32)
        x1c[:, :S] = x1[c * S:(c + 1) * S, :].T
        x0dc = x0c * degc_p[None, :]
        x1dc = x1c * degc_p[None, :]
        in_maps.append({
            "x0T": bf16(x0c), "x1T": bf16(x1c),
            "x0dT": bf16(x0dc), "x1dT": bf16(x1dc),
            "wl0": bf16(inputs["Wl0"]), "wr0": bf16(inputs["Wr0"]),
            "wl1": bf16(inputs["Wl1"]), "wr1": bf16(inputs["Wr1"]),
            "wlm": bf16(inputs["Wlm"]), "wrm": bf16(inputs["Wrm"]),
            "wlo": bf16(inputs["Wlo"]), "wro": bf16(inputs["Wro"]),
            "b01": bf16(np.concatenate([np.asarray(inputs["b0"], np.float32),
                                        np.asarray(inputs["b1"], np.float32)])[None, :]),
            "bm": bf16(np.asarray(inputs["bm"], np.float32)[None, :]),
            "bo": bf16(np.asarray(inputs["bo"], np.float32)[None, :]),
            "idx": idx_img, "dstl": dstloc,
            "invr": bf16(np.broadcast_to(invd_p[None, :], (128, SP))),
            "degr": bf16(np.broadcast_to(degc_p[None, :], (128, SP))),
            "invc": np.ascontiguousarray(invd_p[:NBLK * 128].reshape(NBLK, 128).T,
                                         np.float32),
        })
    return in_maps


def _get_program(edge_index):
    if "prog" in _CACHE:
        return _CACHE["prog"]
    pre = _preprocess(edge_index)
    calls, block_pairs, oh_rng, pair_list, mth = _build_callplan(pre)
    nc = _build_bass(pre, calls, block_pairs, oh_rng, len(pair_list), mth)
    _CACHE["prog"] = (nc, pre, calls, pair_list)
    return _CACHE["prog"]


LAST_EXEC_NS = None


def kernel(**inputs):
    global LAST_EXEC_NS
    _install_hooks()
    from concourse.bass_utils import run_bass_kernel_spmd

    nc, pre, calls, pair_list = _get_program(inputs["edge_index"])
    in_maps = _make_inmaps(inputs, pre, calls, pair_list)
    trace = os.environ.get("KERNEL_TRACE", "0") == "1"
    res = run_bass_kernel_spmd(nc, in_maps, list(range(NCORES)), trace=trace)
    LAST_EXEC_NS = res.exec_time_ns
    return np.concatenate([np.asarray(res.results[c]["out"]) for c in range(NCORES)], axis=0)


# revision 21
# speedup vs baseline: 1.1241x; 1.1241x over previous
"""Self-contained Trainium2 Bass kernel for 4-layer GraphSAGE (nn_LASAGE).

Strategy (v8 - packed 1920-idx gather calls, single_packet=False):
  - Nodes dst-sharded across 8 cores (6250/core, padded to 6272 = 49 blocks of 128).
  - Aggregation is done POST-matmul: agg(x)@Wl == agg(x@Wl), so per layer each
    core computes y = h @ Wl for its own shard; the full Y table [50176, d] is
    replicated via THREE chunked AllGathers (blocks [0:22) [22:39) [39:49)),
    kicked as soon as each chunk's rows are written. Edges gather y[src] rows
    with dma_gather (fp8e4, 256B rows) on 4 SWDGE queues.
  - Gather calls carry up to 15 tiles (1920 idxs) each: single_packet=False
    avoids the 16KB SDMA packet-coalescing limit that capped calls at 1024
    idxs (and wedges the device beyond it). Calls PACK ACROSS dst-block
    boundaries within a chunk's edge stream (tiles may straddle blocks; a
    straddling tile gets one one-hot matmul per block it spans), cutting the
    GpSimd descriptor-gen ucode cost (~994ns fixed + ~2.3ns/idx, the kernel's
    main bottleneck) from ~600 calls to ~171.
  - Per (chunk, block) the edge segment is padded to the max count over cores
    (SPMD: one program, per-core idx/dstl images) with idx=0 / dstcol=-1.
  - Scatter-add into dst blocks via one-hot matmuls on the PE. One-hots are
    {0,1} BF16, built per (block, chunk) with a single DVE IS_EQ against a
    host-provided bf16 dst-column map; matmuls mix fp8 gather tiles with bf16
    one-hots (legal: only fp32 operands must match).
  - Mean-normalization moves to the edges:
      out = invd[dst] * (gather_sum + degc[dst]*(x@Wr) + degc[dst]*b)
    with degc = max(deg,1) pre-scaled Wr inputs and an invd epilogue.
  - All dense operands (x, weights, h storage) are bf16; psum stays fp32.
  - Layer1 fuses conv0+conv1 (concat -> 256 feat). Layer3 (output, d=64) uses
    non-transposed psum (lhsT=onehot) so rows DMA straight to the output;
    its fp8 table rows are 256-wide with only cols 0:64 valid.
  - Rejected on measurement: 2560+/3840-idx calls (per-idx ucode cost rises,
    drain-paced), fp8 DoubleRow scatter matmuls (PE is column-throughput
    bound, no win), 4-chunk AG layout, cross-layer gather preflight (stalls
    on the next table's AllGather mid-loop), indirect_dma_start (HW path
    returns garbage on this runtime).
"""
import sys, os, types

sys.path.insert(0, "/opt/trn_rl_repo")
import numpy as np

N = 50000
E = 800000
NCORES = 8
S = N // NCORES            # 6250 real nodes per core
SP = 6272                  # padded (49 blocks of 128)
NBLK = SP // 128
D1 = 256                   # concat(h0, h1)
DM = 256
DO = 64
MAXI = int(os.environ.get("K_MAXI", "1920"))   # max idxs per dma_gather call
TPC = MAXI // 128          # tiles per full call
CPC = MAXI // 16           # idx-image cols per call
SINGLE_PACKET = os.environ.get("K_SP", "0") == "1"   # >1024 idxs needs False
DOUBLE_ROW = os.environ.get("K_DR", "0") == "1"
NCH = 3
CBLK = [0, 22, 39, 49]       # chunk boundaries in blocks (small tail AG)
CST = [b * 128 for b in CBLK[:-1]]              # chunk start rows (per core)
CSZ = [(CBLK[i + 1] - CBLK[i]) * 128 for i in range(NCH)]   # [2816, 2176, 1280]
TBL = [NCORES * s for s in CSZ]                 # AG table rows (int16-safe)
DMA_SCRATCH = int(os.environ.get("K_SCRATCH", "16384"))  # SWDGE ring carveout


def _install_hooks():
    """antenv.axon_hooks shim so trace=True works in this image (optional)."""
    try:
        import antenv
        if "antenv.axon_hooks" not in sys.modules:
            mod = types.ModuleType("antenv.axon_hooks")
            mod._hook = None
            mod.set_axon_ntff_profile_hook = lambda h: setattr(mod, "_hook", h)
            mod.get_axon_ntff_profile_hook = lambda: mod._hook
            sys.modules["antenv.axon_hooks"] = mod
            antenv.axon_hooks = mod
        from antenv.axon_hooks import get_axon_ntff_profile_hook, set_axon_ntff_profile_hook
        if get_axon_ntff_profile_hook() is None:
            from trn_agent_boot.trn_boot import _ntff_profile_via_ctypes
            set_axon_ntff_profile_hook(_ntff_profile_via_ctypes("/opt/axon/libaxon_pjrt.so"))
        import concourse.bass_utils as bu
        bu.upload_artifacts = lambda tmpdir: f"file://{tmpdir}"
    except Exception:
        pass


def _preprocess(edge_index):
    """Per-core edge streams grouped by (chunk, dst block), max-padded per
    (chunk, block) so the tile/call structure is identical across cores."""
    src = np.asarray(edge_index[0], np.int64)
    dst = np.asarray(edge_index[1], np.int64)
    core = dst // S
    dl = (dst % S).astype(np.int64)
    blk = dl // 128
    col = dl % 128
    sloc = src % S
    chunk = np.digitize(sloc, CST[1:])
    cst = np.asarray(CST)[chunk]
    csz = np.asarray(CSZ)[chunk]
    grow = (src // S) * csz + (sloc - cst)   # row within its chunk-table

    deg = np.bincount(core * S + dl, minlength=N).reshape(NCORES, S)

    order = np.lexsort((grow, blk, chunk, core))
    core_s, ch_s, blk_s, col_s, row_s = (core[order], chunk[order], blk[order],
                                         col[order], grow[order])

    key = (core_s * NCH + ch_s) * NBLK + blk_s
    counts = np.bincount(key, minlength=NCORES * NCH * NBLK).reshape(NCORES, NCH, NBLK)
    cap_hb = counts.max(axis=0).astype(np.int64)     # [NCH, NBLK] segment capacity

    seg_off = np.zeros((NCH, NBLK), np.int64)
    seg_off[:, 1:] = np.cumsum(cap_hb, axis=1)[:, :-1]
    stream_len = cap_hb.sum(axis=1)                  # [NCH]
    tiles_h = np.ceil(stream_len / 128).astype(np.int64)
    pad_len = tiles_h * 128

    srcpad = np.zeros((NCORES, NCH), dtype=object)
    colpad = np.zeros((NCORES, NCH), dtype=object)
    for c in range(NCORES):
        for h in range(NCH):
            srcpad[c, h] = np.zeros(int(pad_len[h]), np.int64)
            colpad[c, h] = np.full(int(pad_len[h]), -1, np.int64)
    grp = key
    first = np.r_[True, grp[1:] != grp[:-1]]
    gidx = np.arange(len(grp)) - np.maximum.accumulate(np.where(first, np.arange(len(grp)), 0))
    pos = seg_off[ch_s, blk_s] + gidx
    for c in range(NCORES):
        m = core_s == c
        for h in range(NCH):
            mh = m & (ch_s == h)
            p = pos[mh]
            srcpad[c, h][p] = row_s[mh]
            colpad[c, h][p] = col_s[mh]

    return {
        "cap_hb": cap_hb, "seg_off": seg_off, "tiles_h": tiles_h,
        "srcpad": srcpad, "colpad": colpad, "deg": deg,
    }


def _build_callplan(pre):
    """Compile-time plan shared by every core.

    calls[ci] = dict(h, k, tile_base, b_first)  — up to TPC tiles of chunk h.
    pairs: for each (block b, chunk h): list of (ci, slot, paircol) where
      paircol indexes the dstl image column for this (tile, block) one-hot.
    dstl columns are ordered by (h, b, tile) so each (b, h)'s columns are
      contiguous: oh_rng[b][h] = (p0, np).
    """
    cap_hb, seg_off, tiles_h = pre["cap_hb"], pre["seg_off"], pre["tiles_h"]
    calls = []
    tile_call = {}           # (h, t) -> (ci, slot)
    for h in range(NCH):
        nt = int(tiles_h[h])
        done = 0
        while done < nt:
            k = min(TPC, nt - done)
            ci = len(calls)
            calls.append(dict(h=h, k=k, tile_base=done, b_first=None))
            for j in range(k):
                tile_call[(h, done + j)] = (ci, j)
            done += k

    # (tile, block) intersections per chunk, ordered by (h, b, t)
    pair_list = []           # global: (h, b, t, ci, slot)
    block_pairs = {b: {h: [] for h in range(NCH)} for b in range(NBLK)}
    oh_rng = {b: {} for b in range(NBLK)}
    for h in range(NCH):
        for b in range(NBLK):
            s0 = int(seg_off[h, b])
            s1 = s0 + int(cap_hb[h, b])
            if s1 == s0:
                oh_rng[b][h] = (len(pair_list), 0)
                continue
            t0, t1 = s0 // 128, (s1 - 1) // 128
            p0 = len(pair_list)
            for t in range(t0, t1 + 1):
                ci, slot = tile_call[(h, t)]
                paircol = len(pair_list)
                pair_list.append((h, b, t, ci, slot))
                block_pairs[b][h].append((ci, slot, paircol))
                if calls[ci]["b_first"] is None:
                    calls[ci]["b_first"] = b
            oh_rng[b][h] = (p0, len(pair_list) - p0)
    for cl in calls:
        if cl["b_first"] is None:       # tail-pad-only call
            cl["b_first"] = NBLK - 1
    # round-robin queues
    for i, cl in enumerate(calls):
        cl["q"] = i % 4
    mth = [max((oh_rng[b][h][1] for b in range(NBLK)), default=1) or 1
           for h in range(NCH)]
    return calls, block_pairs, oh_rng, pair_list, mth


def _idx_arrays(pre, calls, pair_list, core):
    """int16 idx image [128, ncalls*CPC] and per-pair dst-col map (bf16)."""
    import ml_dtypes as _ml
    ncalls = len(calls)
    idx_img = np.zeros((16, ncalls * CPC), np.int16)
    npairs = len(pair_list)
    dstloc = np.full((128, npairs), -1, np.int64)
    cap_hb, seg_off = pre["cap_hb"], pre["seg_off"]
    counts = None
    for ci, cl in enumerate(calls):
        h, k, tb = cl["h"], cl["k"], cl["tile_base"]
        e0 = tb * 128
        nidx = k * 128
        seg_src = pre["srcpad"][core, h][e0:e0 + nidx]
        idx_img[:, ci * CPC: ci * CPC + (nidx // 16)] = seg_src.reshape(-1, 16).T.astype(np.int16)
    # per-core column maps: valid only inside this core's real count range
    # (colpad already holds -1 at padded positions)
    for paircol, (h, b, t, ci, slot) in enumerate(pair_list):
        seg_col = pre["colpad"][core, h][t * 128:(t + 1) * 128]
        s0 = int(seg_off[h, b])
        s1 = s0 + int(cap_hb[h, b])
        j = np.arange(t * 128, (t + 1) * 128)
        inblk = (j >= s0) & (j < s1)
        dstloc[:, paircol] = np.where(inblk, seg_col, -1)
    return np.tile(idx_img, (8, 1)), dstloc.astype(np.float32).astype(_ml.bfloat16)


def _build_bass(pre, calls, block_pairs, oh_rng, npairs, mth):
    import concourse.bass as bass
    import concourse.bacc as bacc
    import concourse.mybir as mybir
    import concourse.tile as tile

    FP32 = mybir.dt.float32
    BF16 = mybir.dt.bfloat16
    F8 = mybir.dt.float8e4
    I16 = mybir.dt.int16
    AL = mybir.AluOpType
    AF = mybir.ActivationFunctionType

    ncalls = len(calls)
    MT = max(mth)            # iota needs to cover the largest oh tile

    nc = bacc.Bacc("TRN2", target_bir_lowering=False, debug=False,
                   enable_asserts=False, num_devices=NCORES, num_swdge_queues=4,
                   dynamic_dma_scratch_size=DMA_SCRATCH)

    x0T = nc.dram_tensor("x0T", [128, SP], BF16, kind="ExternalInput")
    x1T = nc.dram_tensor("x1T", [128, SP], BF16, kind="ExternalInput")
    x0dT = nc.dram_tensor("x0dT", [128, SP], BF16, kind="ExternalInput")
    x1dT = nc.dram_tensor("x1dT", [128, SP], BF16, kind="ExternalInput")
    wl0 = nc.dram_tensor("wl0", [128, 128], BF16, kind="ExternalInput")
    wr0 = nc.dram_tensor("wr0", [128, 128], BF16, kind="ExternalInput")
    wl1 = nc.dram_tensor("wl1", [128, 128], BF16, kind="ExternalInput")
    wr1 = nc.dram_tensor("wr1", [128, 128], BF16, kind="ExternalInput")
    wlm = nc.dram_tensor("wlm", [256, 256], BF16, kind="ExternalInput")
    wrm = nc.dram_tensor("wrm", [256, 256], BF16, kind="ExternalInput")
    wlo = nc.dram_tensor("wlo", [256, 64], BF16, kind="ExternalInput")
    wro = nc.dram_tensor("wro", [256, 64], BF16, kind="ExternalInput")
    b01d = nc.dram_tensor("b01", [1, 256], BF16, kind="ExternalInput")
    bmd = nc.dram_tensor("bm", [1, 256], BF16, kind="ExternalInput")
    bod = nc.dram_tensor("bo", [1, 64], BF16, kind="ExternalInput")
    idxd = nc.dram_tensor("idx", [128, ncalls * CPC], I16, kind="ExternalInput")
    dstld = nc.dram_tensor("dstl", [128, npairs], BF16, kind="ExternalInput")
    invrd = nc.dram_tensor("invr", [128, SP], BF16, kind="ExternalInput")
    degrd = nc.dram_tensor("degr", [128, SP], BF16, kind="ExternalInput")
    invcd = nc.dram_tensor("invc", [128, NBLK], FP32, kind="ExternalInput")
    outd = nc.dram_tensor("out", [S, DO], FP32, kind="ExternalOutput")

    with tile.TileContext(nc) as tc:
        with (
            tc.tile_pool(name="const", bufs=1) as cp,
            tc.tile_pool(name="acts", bufs=1) as hp,
            tc.tile_pool(name="g", bufs=13) as gp,
            tc.tile_pool(name="oh", bufs=2) as ohp,
            tc.tile_pool(name="xs", bufs=6) as xsp,
            # PSUM budget (8 banks): ps0/ps1 (3 bufs each = 6 banks) + py (2)
            tc.tile_pool(name="ps", bufs=3, space="PSUM") as psp,
            tc.tile_pool(name="psy", bufs=2, space="PSUM") as psyp,
            tc.tile_pool(name="ev", bufs=6) as evp,
            tc.tile_pool(name="dram", bufs=1, space="DRAM") as dp,
        ):
            def load(name, dt_, shape, src):
                t = cp.tile(shape, dt_, name=name)
                nc.sync.dma_start(out=t[:], in_=src)
                return t

            wl0t = load("wl0t", BF16, [128, 128], wl0[:])
            wr0t = load("wr0t", BF16, [128, 128], wr0[:])
            wl1t = load("wl1t", BF16, [128, 128], wl1[:])
            wr1t = load("wr1t", BF16, [128, 128], wr1[:])
            wlmt = [load(f"wlmt{i}", BF16, [128, 256], wlm[i * 128:(i + 1) * 128, :]) for i in range(2)]
            wrmt = [load(f"wrmt{i}", BF16, [128, 256], wrm[i * 128:(i + 1) * 128, :]) for i in range(2)]
            wlot = [load(f"wlot{i}", BF16, [128, 64], wlo[i * 128:(i + 1) * 128, :]) for i in range(2)]
            wrot = [load(f"wrot{i}", BF16, [128, 64], wro[i * 128:(i + 1) * 128, :]) for i in range(2)]
            b01t = load("b01t", BF16, [1, 256], b01d[:])
            bmt = load("bmt", BF16, [1, 256], bmd[:])
            bot = load("bot", BF16, [1, 64], bod[:])
            idxt = load("idxt", I16, [128, ncalls * CPC], idxd[:])
            invr = load("invrt", BF16, [128, SP], invrd[:])
            degr = load("degrt", BF16, [128, SP], degrd[:])
            invc = load("invct", FP32, [128, NBLK], invcd[:])
            dstl = load("dstlt", BF16, [128, npairs], dstld[:])

            iota_i = cp.tile([128, MT, 128], mybir.dt.int32, name="iota_i")
            nc.gpsimd.iota(iota_i[:], pattern=[[0, MT], [1, 128]], base=0,
                           channel_multiplier=0)
            iota_bf = cp.tile([128, MT, 128], BF16, name="iota_bf")
            nc.vector.tensor_copy(out=iota_bf[:], in_=iota_i[:])

            warm_own = dp.tile([8, 256], F8, name="warm_own")
            warm_tab = dp.tile([64, 256], F8, name="warm_tab",
                               addr_space="Shared" if NCORES > 4 else "Local")
            wz = evp.tile([8, 256], F8, name="wz", tag="wz")
            nc.vector.memset(wz[:], 0.0)
            nc.sync.dma_start(out=warm_own[:], in_=wz[:])
            nc.gpsimd.collective_compute(
                "AllGather", AL.bypass, replica_groups=[list(range(NCORES))],
                ins=[warm_own[:]], outs=[warm_tab[:]])

            hT = [hp.tile([128, SP], BF16, name=f"hT{i}") for i in range(2)]
            h2T = [hp.tile([128, SP], BF16, name=f"h2T{i}") for i in range(2)]

            shared = "Shared" if NCORES > 4 else "Local"

            def mk_tables(name, width):
                own = [dp.tile([CSZ[h], width], F8, name=f"{name}_own{h}")
                       for h in range(NCH)]
                tab = [dp.tile([TBL[h], width], F8, name=f"{name}{h}",
                               addr_space=shared) for h in range(NCH)]
                return own, tab

            y01_own, Y01 = mk_tables("y01", D1)
            ym_own, Ym = mk_tables("ym", DM)
            yo_own, Yo = mk_tables("yo", 256)

            def chunk_of_block(b):
                for h in range(NCH):
                    if b < CBLK[h + 1]:
                        return h
                raise ValueError(b)

            def write_y(dsts, b, src_tile, dcols):
                h = chunk_of_block(b)
                r0 = b * 128 - CST[h]
                nc.sync.dma_start(out=dsts[h][r0:r0 + 128, 0:dcols],
                                  in_=src_tile[:, 0:dcols])

            RG = [list(range(NCORES))]

            def blk_sl(b):
                return slice(b * 128, (b + 1) * 128)

            def make_ags(own, tab):
                def mk(h):
                    def f():
                        nc.gpsimd.collective_compute(
                            "AllGather", AL.bypass, replica_groups=RG,
                            ins=[own[h][:]], outs=[tab[h][:]])
                    return f
                return [mk(h) for h in range(NCH)]

            def load_oh(b):
                """Build this block's one-hot tiles with a single DVE IS_EQ
                per chunk (bf16 out -> 2x DVE mode)."""
                tiles = {}
                for h in range(NCH):
                    start, nt = oh_rng[b][h]
                    if nt == 0:
                        tiles[h] = (None, start)
                        continue
                    t = ohp.tile([128, mth[h], 128], BF16, name=f"ohb{h}",
                                 tag=f"oh{h}")
                    nc.vector.tensor_tensor(
                        out=t[:, 0:nt, :], in0=iota_bf[:, 0:nt, :],
                        in1=dstl[:, start:start + nt].to_broadcast([128, nt, 128]),
                        op=AL.is_equal)
                    tiles[h] = (t, start)
                return tiles

            # AG kick: chunk i kicks a few blocks after its rows are written
            # so the kick's input-wait is already satisfied; tail at loop end.
            ag_at = {CBLK[1] + 3: [0], CBLK[2] + 3: [1], CBLK[3] - 1: [2]}

            _pg_cache = {}

            def pair_groups(b):
                """Group block b's (tile, block) one-hot pairs for DoubleRow:
                two consecutive entries from the same call with adjacent slots
                and adjacent one-hot columns form one fp8 DoubleRow matmul."""
                if b in _pg_cache:
                    return _pg_cache[b]
                flat = [(h, p) for h in range(NCH) for p in block_pairs[b][h]]
                groups = []
                i = 0
                while i < len(flat):
                    if (DOUBLE_ROW and i + 1 < len(flat)
                            and flat[i][0] == flat[i + 1][0]
                            and flat[i][1][0] == flat[i + 1][1][0]
                            and flat[i][1][1] + 1 == flat[i + 1][1][1]
                            and flat[i][1][2] + 1 == flat[i + 1][1][2]):
                        groups.append([flat[i], flat[i + 1]])
                        i += 2
                    else:
                        groups.append([flat[i]])
                        i += 1
                out = [(g, gi == len(groups) - 1) for gi, g in enumerate(groups)]
                _pg_cache[b] = out
                return out

            # gather emission: per-chunk pointers; chunk 0/1 get a deep
            # lookahead window, chunk 2 a shallow one (its AG lands during
            # the layer's first blocks).
            WCH = [8, 6, 2]
            WPF = [0, None, None]       # cross-layer preflight windows
            PF_B = 99                   # preflight disabled (regressed on HW)
            calls_by_h = {h: [ci for ci, cl in enumerate(calls) if cl["h"] == h]
                          for h in range(NCH)}

            def new_gst():
                return {"ptr": {h: 0 for h in range(NCH)}, "g": {}}

            def emit_for(gst, Ytab, b, wch):
                for h in range(NCH):
                    w = wch[h]
                    if w is None:
                        continue
                    lst = calls_by_h[h]
                    while (gst["ptr"][h] < len(lst)
                           and calls[lst[gst["ptr"][h]]]["b_first"] <= b + w):
                        ci = lst[gst["ptr"][h]]
                        cl = calls[ci]
                        k = cl["k"]
                        g = gp.tile([128, TPC, 256], F8, name="g", tag="g")
                        nc.gpsimd.dma_gather(
                            out_ap=g[:, 0:k, :],
                            in_ap=Ytab[h][:],
                            idxs_ap=idxt[:, ci * CPC: ci * CPC + (k * 128) // 16],
                            num_idxs=k * 128, num_idxs_reg=k * 128,
                            elem_size=256, queue_num=cl["q"], single_packet=SINGLE_PACKET)
                        gst["g"][ci] = g
                        gst["ptr"][h] += 1

            gst1, gst2, gst3 = new_gst(), new_gst(), new_gst()

            # ================= L1 pre: y01_own = [x0@Wl0 | x1@Wl1] =========
            ags01 = make_ags(y01_own, Y01)
            ag_at_pre = {CBLK[1] - 1: 0, CBLK[2] - 1: 1, CBLK[3] - 1: 2}
            for b in range(NBLK):
                x0b = xsp.tile([128, 128], BF16, name="x0b", tag="x0b")
                nc.sync.dma_start(out=x0b[:], in_=x0T[:, blk_sl(b)])
                x1b = xsp.tile([128, 128], BF16, name="x1b", tag="x1b")
                nc.sync.dma_start(out=x1b[:], in_=x1T[:, blk_sl(b)])
                py0 = psp.tile([128, 128], FP32, name="py0", tag="ps0")
                py1 = psp.tile([128, 128], FP32, name="py1", tag="ps1")
                nc.tensor.matmul(py0[:], lhsT=x0b[:], rhs=wl0t[:], start=True, stop=True)
                nc.tensor.matmul(py1[:], lhsT=x1b[:], rhs=wl1t[:], start=True, stop=True)
                evy = evp.tile([128, 256], F8, name="evy", tag="evy", padded_shape=[128, 512])
                nc.vector.tensor_copy(out=evy[:, 0:128], in_=py0[:])
                nc.vector.tensor_copy(out=evy[:, 128:256], in_=py1[:])
                write_y(y01_own, b, evy, D1)
                if b in ag_at_pre:
                    ags01[ag_at_pre[b]]()
                if b >= PF_B:
                    emit_for(gst1, Y01, b - PF_B, WPF)

            # ================= aggregation layer (L1/L2) =====================
            def agg_layer(Ytab, wr_tiles, bias_t, h_src, h_dst, wl_next, y_next,
                          d_next, ags_next, gst, pf_gst=None, pf_tab=None):
                gtiles = gst["g"]

                for b in range(NBLK):
                    emit_for(gst, Ytab, b, WCH)
                    ohb = load_oh(b)
                    ps0 = psp.tile([128, 128], FP32, name="ps0", tag="ps0")
                    ps1 = psp.tile([128, 128], FP32, name="ps1", tag="ps1")
                    if h_src is None:
                        x0b = xsp.tile([128, 128], BF16, name="x0b2", tag="xd0")
                        nc.sync.dma_start(out=x0b[:], in_=x0dT[:, blk_sl(b)])
                        x1b = xsp.tile([128, 128], BF16, name="x1b2", tag="xd1")
                        nc.sync.dma_start(out=x1b[:], in_=x1dT[:, blk_sl(b)])
                        nc.tensor.matmul(ps0[:], lhsT=wr0t[:], rhs=x0b[:], start=True, stop=False)
                        nc.tensor.matmul(ps1[:], lhsT=wr1t[:], rhs=x1b[:], start=True, stop=False)
                    else:
                        hd0 = evp.tile([128, 128], BF16, name="hd0", tag="hd0")
                        nc.vector.tensor_tensor(out=hd0[:], in0=h_src[0][:, blk_sl(b)],
                                                in1=degr[:, blk_sl(b)], op=AL.mult)
                        hd1 = evp.tile([128, 128], BF16, name="hd1", tag="hd1")
                        nc.vector.tensor_tensor(out=hd1[:], in0=h_src[1][:, blk_sl(b)],
                                                in1=degr[:, blk_sl(b)], op=AL.mult)
                        nc.tensor.matmul(ps0[:], lhsT=wr_tiles[0][:, 0:128], rhs=hd0[:], start=True, stop=False)
                        nc.tensor.matmul(ps0[:], lhsT=wr_tiles[1][:, 0:128], rhs=hd1[:], start=False, stop=False)
                        nc.tensor.matmul(ps1[:], lhsT=wr_tiles[0][:, 128:256], rhs=hd0[:], start=True, stop=False)
                        nc.tensor.matmul(ps1[:], lhsT=wr_tiles[1][:, 128:256], rhs=hd1[:], start=False, stop=False)
                    nc.tensor.matmul(ps0[:], lhsT=bias_t[0:1, 0:128], rhs=degr[0:1, blk_sl(b)],
                                     start=False, stop=False)
                    nc.tensor.matmul(ps1[:], lhsT=bias_t[0:1, 128:256], rhs=degr[0:1, blk_sl(b)],
                                     start=False, stop=False)
                    for grp, last in pair_groups(b):
                        h, (ci, slot, paircol) = grp[0]
                        g = gtiles[ci]
                        oht, start = ohb[h]
                        j = paircol - start
                        if len(grp) == 2:
                            nc.tensor.matmul(ps0[:], lhsT=g[:, slot:slot + 2, 0:128],
                                             rhs=oht[:, j:j + 2, :], start=False, stop=last,
                                             perf_mode=mybir.MatmulPerfMode.DoubleRow)
                            nc.tensor.matmul(ps1[:], lhsT=g[:, slot:slot + 2, 128:256],
                                             rhs=oht[:, j:j + 2, :], start=False, stop=last,
                                             perf_mode=mybir.MatmulPerfMode.DoubleRow)
                        else:
                            nc.tensor.matmul(ps0[:], lhsT=g[:, slot, 0:128], rhs=oht[:, j, :],
                                             start=False, stop=last)
                            nc.tensor.matmul(ps1[:], lhsT=g[:, slot, 128:256], rhs=oht[:, j, :],
                                             start=False, stop=last)
                    # epilogue: h = relu(ps) * invd  (relu commutes with the
                    # positive per-column scale)
                    rt0 = evp.tile([128, 128], BF16, name="rt0", tag="rt0")
                    nc.scalar.activation(rt0[:], ps0[:], AF.Relu)
                    nc.vector.tensor_tensor(out=h_dst[0][:, blk_sl(b)], in0=rt0[:],
                                            in1=invr[:, blk_sl(b)], op=AL.mult)
                    rt1 = evp.tile([128, 128], BF16, name="rt1", tag="rt1")
                    nc.scalar.activation(rt1[:], ps1[:], AF.Relu)
                    nc.vector.tensor_tensor(out=h_dst[1][:, blk_sl(b)], in0=rt1[:],
                                            in1=invr[:, blk_sl(b)], op=AL.mult)
                    pyn = psyp.tile([128, d_next], FP32, name="pyn", tag="py",
                                    padded_shape=[128, 256])
                    nc.tensor.matmul(pyn[:], lhsT=h_dst[0][:, blk_sl(b)], rhs=wl_next[0][:],
                                     start=True, stop=False)
                    nc.tensor.matmul(pyn[:], lhsT=h_dst[1][:, blk_sl(b)], rhs=wl_next[1][:],
                                     start=False, stop=True)
                    evn = evp.tile([128, d_next], F8, name="evn", tag="evy",
                                   padded_shape=[128, 512])
                    nc.vector.tensor_copy(out=evn[:], in_=pyn[:])
                    write_y(y_next, b, evn, d_next)
                    if b in ag_at:
                        for hh in ag_at[b]:
                            ags_next[hh]()
                    if pf_gst is not None and b >= PF_B:
                        emit_for(pf_gst, pf_tab, b - PF_B, WPF)

            agg_layer(Y01, None, b01t, None, hT, wlmt, ym_own, DM,
                      make_ags(ym_own, Ym), gst1, gst2, Ym)
            agg_layer(Ym, wrmt, bmt, hT, h2T, wlot, yo_own, DO,
                      make_ags(yo_own, Yo), gst2, gst3, Yo)

            # ================= L3: out[node, 64] ============================
            gtiles3 = gst3["g"]

            for b in range(NBLK):
                emit_for(gst3, Yo, b, WCH)
                ohb = load_oh(b)
                ps3 = psp.tile([128, DO], FP32, name="ps3", tag="ps0",
                               padded_shape=[128, 128])
                hd0 = evp.tile([128, 128], BF16, name="hd20", tag="hd0")
                nc.vector.tensor_tensor(out=hd0[:], in0=h2T[0][:, blk_sl(b)],
                                        in1=degr[:, blk_sl(b)], op=AL.mult)
                hd1 = evp.tile([128, 128], BF16, name="hd21", tag="hd1")
                nc.vector.tensor_tensor(out=hd1[:], in0=h2T[1][:, blk_sl(b)],
                                        in1=degr[:, blk_sl(b)], op=AL.mult)
                nc.tensor.matmul(ps3[:], lhsT=hd0[:], rhs=wrot[0][:],
                                 start=True, stop=False)
                nc.tensor.matmul(ps3[:], lhsT=hd1[:], rhs=wrot[1][:],
                                 start=False, stop=False)
                nc.tensor.matmul(ps3[:], lhsT=degr[0:1, blk_sl(b)], rhs=bot[0:1, :],
                                 start=False, stop=False)
                for grp, last in pair_groups(b):
                    h, (ci, slot, paircol) = grp[0]
                    g3 = gtiles3[ci]
                    oht, start = ohb[h]
                    j = paircol - start
                    if len(grp) == 2:
                        nc.tensor.matmul(ps3[:], lhsT=oht[:, j:j + 2, :],
                                         rhs=g3[:, slot:slot + 2, 0:64],
                                         start=False, stop=last,
                                         perf_mode=mybir.MatmulPerfMode.DoubleRow)
                    else:
                        nc.tensor.matmul(ps3[:], lhsT=oht[:, j, :], rhs=g3[:, slot, 0:64],
                                         start=False, stop=last)
                osb = evp.tile([128, DO], FP32, name="osb", tag="osb")
                nc.scalar.activation(osb[:], ps3[:], AF.Copy,
                                     scale=invc[:, b:b + 1])
                rows = min(128, S - b * 128)
                nc.sync.dma_start(out=outd[b * 128: b * 128 + rows, :],
                                  in_=osb[0:rows, :])

    nc.finalize()
    return nc


_CACHE = {}


def _make_inmaps(inputs, pre, calls, pair_list):
    import ml_dtypes as _ml
    BF = _ml.bfloat16
    x0 = np.asarray(inputs["x0"], np.float32)
    x1 = np.asarray(inputs["x1"], np.float32)
    deg = pre["deg"]
    bf16 = lambda a: np.ascontiguousarray(a).astype(BF)
    in_maps = []
    for c in range(NCORES):
        degc = np.maximum(deg[c], 1.0).astype(np.float32)
        invd = (1.0 / degc).astype(np.float32)
        degc_p = np.ones(SP, np.float32)
        degc_p[:S] = degc
        invd_p = np.ones(SP, np.float32)
        invd_p[:S] = invd
        idx_img, dstloc = _idx_arrays(pre, calls, pair_list, c)
        x0c = np.zeros((128, SP), np.float32)
        x0c[:, :S] = x0[c * S:(c + 1) * S, :].T
        x1c = np.zeros((128, SP), np.float32)
        x1c[:, :S] = x1[c * S:(c + 1) * S, :].T
        x0dc = x0c * degc_p[None, :]
        x1dc = x1c * degc_p[None, :]
        in_maps.append({
            "x0T": bf16(x0c), "x1T": bf16(x1c),
            "x0dT": bf16(x0dc), "x1dT": bf16(x1dc),
            "wl0": bf16(inputs["Wl0"]), "wr0": bf16(inputs["Wr0"]),
            "wl1": bf16(inputs["Wl1"]), "wr1": bf16(inputs["Wr1"]),
            "wlm": bf16(inputs["Wlm"]), "wrm": bf16(inputs["Wrm"]),
            "wlo": bf16(inputs["Wlo"]), "wro": bf16(inputs["Wro"]),
            "b01": bf16(np.concatenate([np.asarray(inputs["b0"], np.float32),
                                        np.asarray(inputs["b1"], np.float32)])[None, :]),
            "bm": bf16(np.asarray(inputs["bm"], np.float32)[None, :]),
            "bo": bf16(np.asarray(inputs["bo"], np.float32)[None, :]),
            "idx": idx_img, "dstl": dstloc,
            "invr": bf16(np.broadcast_to(invd_p[None, :], (128, SP))),
            "degr": bf16(np.broadcast_to(degc_p[None, :], (128, SP))),
            "invc": np.ascontiguousarray(invd_p[:NBLK * 128].reshape(NBLK, 128).T,
                                         np.float32),
        })
    return in_maps


def _get_program(edge_index):
    if "prog" in _CACHE:
        return _CACHE["prog"]
    pre = _preprocess(edge_index)
    calls, block_pairs, oh_rng, pair_list, mth = _build_callplan(pre)
    nc = _build_bass(pre, calls, block_pairs, oh_rng, len(pair_list), mth)
    _CACHE["prog"] = (nc, pre, calls, pair_list)
    return _CACHE["prog"]


LAST_EXEC_NS = None


def kernel(**inputs):
    global LAST_EXEC_NS
    _install_hooks()
    from concourse.bass_utils import run_bass_kernel_spmd

    nc, pre, calls, pair_list = _get_program(inputs["edge_index"])
    in_maps = _make_inmaps(inputs, pre, calls, pair_list)
    trace = os.environ.get("KERNEL_TRACE", "0") == "1"
    res = run_bass_kernel_spmd(nc, in_maps, list(range(NCORES)), trace=trace)
    LAST_EXEC_NS = res.exec_time_ns
    return np.concatenate([np.asarray(res.results[c]["out"]) for c in range(NCORES)], axis=0)


# revision 24
# speedup vs baseline: 1.1591x; 1.0311x over previous
"""Self-contained Trainium2 Bass kernel for 4-layer GraphSAGE (nn_LASAGE).

Strategy (v8 - packed 1920-idx gather calls, single_packet=False):
  - Nodes dst-sharded across 8 cores (6250/core, padded to 6272 = 49 blocks of 128).
  - Aggregation is done POST-matmul: agg(x)@Wl == agg(x@Wl), so per layer each
    core computes y = h @ Wl for its own shard; the full Y table [50176, d] is
    replicated via THREE chunked AllGathers (blocks [0:22) [22:39) [39:49)),
    kicked as soon as each chunk's rows are written. Edges gather y[src] rows
    with dma_gather (fp8e4, 256B rows) on 4 SWDGE queues.
  - Gather calls carry up to 15 tiles (1920 idxs) each: single_packet=False
    avoids the 16KB SDMA packet-coalescing limit that capped calls at 1024
    idxs (and wedges the device beyond it). Calls PACK ACROSS dst-block
    boundaries within a chunk's edge stream (tiles may straddle blocks; a
    straddling tile gets one one-hot matmul per block it spans), cutting the
    GpSimd descriptor-gen ucode cost (~994ns fixed + ~2.3ns/idx, the kernel's
    main bottleneck) from ~600 calls to ~171.
  - Per (chunk, block) the edge segment is padded to the max count over cores
    (SPMD: one program, per-core idx/dstl images) with idx=0 / dstcol=-1.
  - Scatter-add into dst blocks via one-hot matmuls on the PE. One-hots are
    {0,1} BF16, built per (block, chunk) with a single DVE IS_EQ against a
    host-provided bf16 dst-column map; matmuls mix fp8 gather tiles with bf16
    one-hots (legal: only fp32 operands must match).
  - Mean-normalization moves to the edges:
      out = invd[dst] * (gather_sum + degc[dst]*(x@Wr) + degc[dst]*b)
    with degc = max(deg,1) pre-scaled Wr inputs and an invd epilogue.
  - All dense operands (x, weights, h storage) are bf16; psum stays fp32.
  - Layer1 fuses conv0+conv1 (concat -> 256 feat). Layer3 (output, d=64) uses
    non-transposed psum (lhsT=onehot) so rows DMA straight to the output;
    its fp8 table rows are 256-wide with only cols 0:64 valid.
  - Rejected on measurement: 2560+/3840-idx calls (per-idx ucode cost rises,
    drain-paced), fp8 DoubleRow scatter matmuls (PE is column-throughput
    bound, no win), 4-chunk AG layout, cross-layer gather preflight (stalls
    on the next table's AllGather mid-loop), indirect_dma_start (HW path
    returns garbage on this runtime).
"""
import sys, os, types

sys.path.insert(0, "/opt/trn_rl_repo")
import numpy as np

N = 50000
E = 800000
NCORES = 8
S = N // NCORES            # 6250 real nodes per core
SP = 6272                  # padded (49 blocks of 128)
NBLK = SP // 128
D1 = 256                   # concat(h0, h1)
DM = 256
DO = 64
MAXI = int(os.environ.get("K_MAXI", "1920"))   # max idxs per dma_gather call
TPC = MAXI // 128          # tiles per full call
CPC = MAXI // 16           # idx-image cols per call
SINGLE_PACKET = os.environ.get("K_SP", "0") == "1"   # >1024 idxs needs False
DOUBLE_ROW = os.environ.get("K_DR", "0") == "1"
NCH = 3
CBLK = [0, 22, 39, 49]       # chunk boundaries in blocks (small tail AG)
CST = [b * 128 for b in CBLK[:-1]]              # chunk start rows (per core)
CSZ = [(CBLK[i + 1] - CBLK[i]) * 128 for i in range(NCH)]   # [2816, 2176, 1280]
TBL = [NCORES * s for s in CSZ]                 # AG table rows (int16-safe)
DMA_SCRATCH = int(os.environ.get("K_SCRATCH", "16384"))  # SWDGE ring carveout


def _install_hooks():
    """antenv.axon_hooks shim so trace=True works in this image (optional)."""
    try:
        import antenv
        if "antenv.axon_hooks" not in sys.modules:
            mod = types.ModuleType("antenv.axon_hooks")
            mod._hook = None
            mod.set_axon_ntff_profile_hook = lambda h: setattr(mod, "_hook", h)
            mod.get_axon_ntff_profile_hook = lambda: mod._hook
            sys.modules["antenv.axon_hooks"] = mod
            antenv.axon_hooks = mod
        from antenv.axon_hooks import get_axon_ntff_profile_hook, set_axon_ntff_profile_hook
        if get_axon_ntff_profile_hook() is None:
            from trn_agent_boot.trn_boot import _ntff_profile_via_ctypes
            set_axon_ntff_profile_hook(_ntff_profile_via_ctypes("/opt/axon/libaxon_pjrt.so"))
        import concourse.bass_utils as bu
        bu.upload_artifacts = lambda tmpdir: f"file://{tmpdir}"
    except Exception:
        pass


def _preprocess(edge_index):
    """Per-core edge streams grouped by (chunk, dst block), max-padded per
    (chunk, block) so the tile/call structure is identical across cores."""
    src = np.asarray(edge_index[0], np.int64)
    dst = np.asarray(edge_index[1], np.int64)
    core = dst // S
    dl = (dst % S).astype(np.int64)
    blk = dl // 128
    col = dl % 128
    sloc = src % S
    chunk = np.digitize(sloc, CST[1:])
    cst = np.asarray(CST)[chunk]
    csz = np.asarray(CSZ)[chunk]
    grow = (src // S) * csz + (sloc - cst)   # row within its chunk-table

    deg = np.bincount(core * S + dl, minlength=N).reshape(NCORES, S)

    order = np.lexsort((grow, blk, chunk, core))
    core_s, ch_s, blk_s, col_s, row_s = (core[order], chunk[order], blk[order],
                                         col[order], grow[order])

    key = (core_s * NCH + ch_s) * NBLK + blk_s
    counts = np.bincount(key, minlength=NCORES * NCH * NBLK).reshape(NCORES, NCH, NBLK)
    cap_hb = counts.max(axis=0).astype(np.int64)     # [NCH, NBLK] segment capacity

    seg_off = np.zeros((NCH, NBLK), np.int64)
    seg_off[:, 1:] = np.cumsum(cap_hb, axis=1)[:, :-1]
    stream_len = cap_hb.sum(axis=1)                  # [NCH]
    tiles_h = np.ceil(stream_len / 128).astype(np.int64)
    pad_len = tiles_h * 128

    srcpad = np.zeros((NCORES, NCH), dtype=object)
    colpad = np.zeros((NCORES, NCH), dtype=object)
    for c in range(NCORES):
        for h in range(NCH):
            srcpad[c, h] = np.zeros(int(pad_len[h]), np.int64)
            colpad[c, h] = np.full(int(pad_len[h]), -1, np.int64)
    grp = key
    first = np.r_[True, grp[1:] != grp[:-1]]
    gidx = np.arange(len(grp)) - np.maximum.accumulate(np.where(first, np.arange(len(grp)), 0))
    pos = seg_off[ch_s, blk_s] + gidx
    for c in range(NCORES):
        m = core_s == c
        for h in range(NCH):
            mh = m & (ch_s == h)
            p = pos[mh]
            srcpad[c, h][p] = row_s[mh]
            colpad[c, h][p] = col_s[mh]

    return {
        "cap_hb": cap_hb, "seg_off": seg_off, "tiles_h": tiles_h,
        "srcpad": srcpad, "colpad": colpad, "deg": deg,
    }


def _build_callplan(pre):
    """Compile-time plan shared by every core.

    calls[ci] = dict(h, k, tile_base, b_first)  — up to TPC tiles of chunk h.
    pairs: for each (block b, chunk h): list of (ci, slot, paircol) where
      paircol indexes the dstl image column for this (tile, block) one-hot.
    dstl columns are ordered by (h, b, tile) so each (b, h)'s columns are
      contiguous: oh_rng[b][h] = (p0, np).
    """
    cap_hb, seg_off, tiles_h = pre["cap_hb"], pre["seg_off"], pre["tiles_h"]
    calls = []
    tile_call = {}           # (h, t) -> (ci, slot)
    for h in range(NCH):
        nt = int(tiles_h[h])
        done = 0
        while done < nt:
            k = min(TPC, nt - done)
            ci = len(calls)
            calls.append(dict(h=h, k=k, tile_base=done, b_first=None))
            for j in range(k):
                tile_call[(h, done + j)] = (ci, j)
            done += k

    # (tile, block) intersections per chunk, ordered by (h, b, t)
    pair_list = []           # global: (h, b, t, ci, slot)
    block_pairs = {b: {h: [] for h in range(NCH)} for b in range(NBLK)}
    oh_rng = {b: {} for b in range(NBLK)}
    for h in range(NCH):
        for b in range(NBLK):
            s0 = int(seg_off[h, b])
            s1 = s0 + int(cap_hb[h, b])
            if s1 == s0:
                oh_rng[b][h] = (len(pair_list), 0)
                continue
            t0, t1 = s0 // 128, (s1 - 1) // 128
            p0 = len(pair_list)
            for t in range(t0, t1 + 1):
                ci, slot = tile_call[(h, t)]
                paircol = len(pair_list)
                pair_list.append((h, b, t, ci, slot))
                block_pairs[b][h].append((ci, slot, paircol))
                if calls[ci]["b_first"] is None:
                    calls[ci]["b_first"] = b
            oh_rng[b][h] = (p0, len(pair_list) - p0)
    for cl in calls:
        if cl["b_first"] is None:       # tail-pad-only call
            cl["b_first"] = NBLK - 1
    # round-robin queues
    for i, cl in enumerate(calls):
        cl["q"] = i % 4
    mth = [max((oh_rng[b][h][1] for b in range(NBLK)), default=1) or 1
           for h in range(NCH)]
    return calls, block_pairs, oh_rng, pair_list, mth


def _idx_arrays(pre, calls, pair_list, core):
    """int16 idx image [128, ncalls*CPC] and per-pair dst-col map (bf16)."""
    import ml_dtypes as _ml
    ncalls = len(calls)
    idx_img = np.zeros((16, ncalls * CPC), np.int16)
    npairs = len(pair_list)
    dstloc = np.full((128, npairs), -1, np.int64)
    cap_hb, seg_off = pre["cap_hb"], pre["seg_off"]
    counts = None
    for ci, cl in enumerate(calls):
        h, k, tb = cl["h"], cl["k"], cl["tile_base"]
        e0 = tb * 128
        nidx = k * 128
        seg_src = pre["srcpad"][core, h][e0:e0 + nidx]
        idx_img[:, ci * CPC: ci * CPC + (nidx // 16)] = seg_src.reshape(-1, 16).T.astype(np.int16)
    # per-core column maps: valid only inside this core's real count range
    # (colpad already holds -1 at padded positions)
    for paircol, (h, b, t, ci, slot) in enumerate(pair_list):
        seg_col = pre["colpad"][core, h][t * 128:(t + 1) * 128]
        s0 = int(seg_off[h, b])
        s1 = s0 + int(cap_hb[h, b])
        j = np.arange(t * 128, (t + 1) * 128)
        inblk = (j >= s0) & (j < s1)
        dstloc[:, paircol] = np.where(inblk, seg_col, -1)
    return np.tile(idx_img, (8, 1)), dstloc.astype(np.float32).astype(_ml.bfloat16)


def _build_bass(pre, calls, block_pairs, oh_rng, npairs, mth):
    import concourse.bass as bass
    import concourse.bacc as bacc
    import concourse.mybir as mybir
    import concourse.tile as tile

    FP32 = mybir.dt.float32
    BF16 = mybir.dt.bfloat16
    F8 = mybir.dt.float8e4
    I16 = mybir.dt.int16
    AL = mybir.AluOpType
    AF = mybir.ActivationFunctionType

    ncalls = len(calls)
    MT = max(mth)            # iota needs to cover the largest oh tile

    nc = bacc.Bacc("TRN2", target_bir_lowering=False, debug=False,
                   enable_asserts=False, num_devices=NCORES, num_swdge_queues=4,
                   dynamic_dma_scratch_size=DMA_SCRATCH)

    x0T = nc.dram_tensor("x0T", [128, SP], BF16, kind="ExternalInput")
    x1T = nc.dram_tensor("x1T", [128, SP], BF16, kind="ExternalInput")
    x0dT = nc.dram_tensor("x0dT", [128, SP], BF16, kind="ExternalInput")
    x1dT = nc.dram_tensor("x1dT", [128, SP], BF16, kind="ExternalInput")
    wl0 = nc.dram_tensor("wl0", [128, 128], BF16, kind="ExternalInput")
    wr0 = nc.dram_tensor("wr0", [128, 128], BF16, kind="ExternalInput")
    wl1 = nc.dram_tensor("wl1", [128, 128], BF16, kind="ExternalInput")
    wr1 = nc.dram_tensor("wr1", [128, 128], BF16, kind="ExternalInput")
    wlm = nc.dram_tensor("wlm", [256, 256], BF16, kind="ExternalInput")
    wrm = nc.dram_tensor("wrm", [256, 256], BF16, kind="ExternalInput")
    wlo = nc.dram_tensor("wlo", [256, 64], BF16, kind="ExternalInput")
    wro = nc.dram_tensor("wro", [256, 64], BF16, kind="ExternalInput")
    b01d = nc.dram_tensor("b01", [1, 256], BF16, kind="ExternalInput")
    bmd = nc.dram_tensor("bm", [1, 256], BF16, kind="ExternalInput")
    bod = nc.dram_tensor("bo", [1, 64], BF16, kind="ExternalInput")
    idxd = nc.dram_tensor("idx", [128, ncalls * CPC], I16, kind="ExternalInput")
    dstld = nc.dram_tensor("dstl", [128, npairs], BF16, kind="ExternalInput")
    invrd = nc.dram_tensor("invr", [128, SP], BF16, kind="ExternalInput")
    degrd = nc.dram_tensor("degr", [128, SP], BF16, kind="ExternalInput")
    invcd = nc.dram_tensor("invc", [128, NBLK], FP32, kind="ExternalInput")
    outd = nc.dram_tensor("out", [S, DO], FP32, kind="ExternalOutput")

    with tile.TileContext(nc) as tc:
        with (
            tc.tile_pool(name="const", bufs=1) as cp,
            tc.tile_pool(name="acts", bufs=1) as hp,
            tc.tile_pool(name="g", bufs=15) as gp,
            tc.tile_pool(name="oh", bufs=2) as ohp,
            tc.tile_pool(name="xs", bufs=6) as xsp,
            # PSUM budget (8 banks): ps0/ps1 (3 bufs each = 6 banks) + py (2)
            tc.tile_pool(name="ps", bufs=3, space="PSUM") as psp,
            tc.tile_pool(name="psy", bufs=2, space="PSUM") as psyp,
            tc.tile_pool(name="ev", bufs=6) as evp,
            tc.tile_pool(name="dram", bufs=1, space="DRAM") as dp,
        ):
            def load(name, dt_, shape, src):
                t = cp.tile(shape, dt_, name=name)
                nc.sync.dma_start(out=t[:], in_=src)
                return t

            wl0t = load("wl0t", BF16, [128, 128], wl0[:])
            wr0t = load("wr0t", BF16, [128, 128], wr0[:])
            wl1t = load("wl1t", BF16, [128, 128], wl1[:])
            wr1t = load("wr1t", BF16, [128, 128], wr1[:])
            wlmt = [load(f"wlmt{i}", BF16, [128, 256], wlm[i * 128:(i + 1) * 128, :]) for i in range(2)]
            wrmt = [load(f"wrmt{i}", BF16, [128, 256], wrm[i * 128:(i + 1) * 128, :]) for i in range(2)]
            wlot = [load(f"wlot{i}", BF16, [128, 64], wlo[i * 128:(i + 1) * 128, :]) for i in range(2)]
            wrot = [load(f"wrot{i}", BF16, [128, 64], wro[i * 128:(i + 1) * 128, :]) for i in range(2)]
            b01t = load("b01t", BF16, [1, 256], b01d[:])
            bmt = load("bmt", BF16, [1, 256], bmd[:])
            bot = load("bot", BF16, [1, 64], bod[:])
            idxt = load("idxt", I16, [128, ncalls * CPC], idxd[:])
            invr = load("invrt", BF16, [128, SP], invrd[:])
            degr = load("degrt", BF16, [128, SP], degrd[:])
            invc = load("invct", FP32, [128, NBLK], invcd[:])
            dstl = load("dstlt", BF16, [128, npairs], dstld[:])

            iota_i = cp.tile([128, MT, 128], mybir.dt.int32, name="iota_i")
            nc.gpsimd.iota(iota_i[:], pattern=[[0, MT], [1, 128]], base=0,
                           channel_multiplier=0)
            iota_bf = cp.tile([128, MT, 128], BF16, name="iota_bf")
            nc.vector.tensor_copy(out=iota_bf[:], in_=iota_i[:])

            warm_own = dp.tile([8, 256], F8, name="warm_own")
            warm_tab = dp.tile([64, 256], F8, name="warm_tab",
                               addr_space="Shared" if NCORES > 4 else "Local")
            wz = evp.tile([8, 256], F8, name="wz", tag="wz")
            nc.vector.memset(wz[:], 0.0)
            nc.sync.dma_start(out=warm_own[:], in_=wz[:])
            nc.gpsimd.collective_compute(
                "AllGather", AL.bypass, replica_groups=[list(range(NCORES))],
                ins=[warm_own[:]], outs=[warm_tab[:]])

            hT = [hp.tile([128, SP], BF16, name=f"hT{i}") for i in range(2)]
            h2T = [hp.tile([128, SP], BF16, name=f"h2T{i}") for i in range(2)]

            shared = "Shared" if NCORES > 4 else "Local"

            def mk_tables(name, width):
                own = [dp.tile([CSZ[h], width], F8, name=f"{name}_own{h}")
                       for h in range(NCH)]
                tab = [dp.tile([TBL[h], width], F8, name=f"{name}{h}",
                               addr_space=shared) for h in range(NCH)]
                return own, tab

            y01_own, Y01 = mk_tables("y01", D1)
            ym_own, Ym = mk_tables("ym", DM)
            yo_own, Yo = mk_tables("yo", 256)

            def chunk_of_block(b):
                for h in range(NCH):
                    if b < CBLK[h + 1]:
                        return h
                raise ValueError(b)

            def write_y(dsts, b, src_tile, dcols):
                h = chunk_of_block(b)
                r0 = b * 128 - CST[h]
                nc.sync.dma_start(out=dsts[h][r0:r0 + 128, 0:dcols],
                                  in_=src_tile[:, 0:dcols])

            RG = [list(range(NCORES))]

            def blk_sl(b):
                return slice(b * 128, (b + 1) * 128)

            def make_ags(own, tab):
                def mk(h):
                    def f():
                        nc.gpsimd.collective_compute(
                            "AllGather", AL.bypass, replica_groups=RG,
                            ins=[own[h][:]], outs=[tab[h][:]])
                    return f
                return [mk(h) for h in range(NCH)]

            def load_oh(b):
                """Build this block's one-hot tiles with a single DVE IS_EQ
                per chunk (bf16 out -> 2x DVE mode)."""
                tiles = {}
                for h in range(NCH):
                    start, nt = oh_rng[b][h]
                    if nt == 0:
                        tiles[h] = (None, start)
                        continue
                    t = ohp.tile([128, mth[h], 128], BF16, name=f"ohb{h}",
                                 tag=f"oh{h}")
                    nc.vector.tensor_tensor(
                        out=t[:, 0:nt, :], in0=iota_bf[:, 0:nt, :],
                        in1=dstl[:, start:start + nt].to_broadcast([128, nt, 128]),
                        op=AL.is_equal)
                    tiles[h] = (t, start)
                return tiles

            # AG kick: chunk i kicks a few blocks after its rows are written
            # so the kick's input-wait is already satisfied; tail at loop end.
            ag_at = {CBLK[1] + 3: [0], CBLK[2] + 3: [1], CBLK[3] - 1: [2]}

            _pg_cache = {}

            def pair_groups(b):
                """Group block b's (tile, block) one-hot pairs for DoubleRow:
                two consecutive entries from the same call with adjacent slots
                and adjacent one-hot columns form one fp8 DoubleRow matmul."""
                if b in _pg_cache:
                    return _pg_cache[b]
                flat = [(h, p) for h in range(NCH) for p in block_pairs[b][h]]
                groups = []
                i = 0
                while i < len(flat):
                    if (DOUBLE_ROW and i + 1 < len(flat)
                            and flat[i][0] == flat[i + 1][0]
                            and flat[i][1][0] == flat[i + 1][1][0]
                            and flat[i][1][1] + 1 == flat[i + 1][1][1]
                            and flat[i][1][2] + 1 == flat[i + 1][1][2]):
                        groups.append([flat[i], flat[i + 1]])
                        i += 2
                    else:
                        groups.append([flat[i]])
                        i += 1
                out = [(g, gi == len(groups) - 1) for gi, g in enumerate(groups)]
                _pg_cache[b] = out
                return out

            # gather emission: per-chunk pointers; chunk 0/1 get a deep
            # lookahead window, chunk 2 a shallow one (its AG lands during
            # the layer's first blocks).
            WCH = [10, 6, 2]
            WPF = [0, None, None]       # cross-layer preflight windows
            PF_B = 99                   # preflight disabled (regressed on HW)
            calls_by_h = {h: [ci for ci, cl in enumerate(calls) if cl["h"] == h]
                          for h in range(NCH)}

            def new_gst():
                return {"ptr": {h: 0 for h in range(NCH)}, "g": {}}

            def emit_for(gst, Ytab, b, wch):
                for h in range(NCH):
                    w = wch[h]
                    if w is None:
                        continue
                    lst = calls_by_h[h]
                    while (gst["ptr"][h] < len(lst)
                           and calls[lst[gst["ptr"][h]]]["b_first"] <= b + w):
                        ci = lst[gst["ptr"][h]]
                        cl = calls[ci]
                        k = cl["k"]
                        g = gp.tile([128, TPC, 256], F8, name="g", tag="g")
                        nc.gpsimd.dma_gather(
                            out_ap=g[:, 0:k, :],
                            in_ap=Ytab[h][:],
                            idxs_ap=idxt[:, ci * CPC: ci * CPC + (k * 128) // 16],
                            num_idxs=k * 128, num_idxs_reg=k * 128,
                            elem_size=256, queue_num=cl["q"], single_packet=SINGLE_PACKET)
                        gst["g"][ci] = g
                        gst["ptr"][h] += 1

            gst1, gst2, gst3 = new_gst(), new_gst(), new_gst()

            # ================= L1 pre: y01_own = [x0@Wl0 | x1@Wl1] =========
            ags01 = make_ags(y01_own, Y01)
            ag_at_pre = {CBLK[1] - 1: 0, CBLK[2] - 1: 1, CBLK[3] - 1: 2}
            for b in range(NBLK):
                x0b = xsp.tile([128, 128], BF16, name="x0b", tag="x0b")
                nc.sync.dma_start(out=x0b[:], in_=x0T[:, blk_sl(b)])
                x1b = xsp.tile([128, 128], BF16, name="x1b", tag="x1b")
                nc.sync.dma_start(out=x1b[:], in_=x1T[:, blk_sl(b)])
                py0 = psp.tile([128, 128], FP32, name="py0", tag="ps0")
                py1 = psp.tile([128, 128], FP32, name="py1", tag="ps1")
                nc.tensor.matmul(py0[:], lhsT=x0b[:], rhs=wl0t[:], start=True, stop=True)
                nc.tensor.matmul(py1[:], lhsT=x1b[:], rhs=wl1t[:], start=True, stop=True)
                evy = evp.tile([128, 256], F8, name="evy", tag="evy", padded_shape=[128, 512])
                nc.vector.tensor_copy(out=evy[:, 0:128], in_=py0[:])
                nc.vector.tensor_copy(out=evy[:, 128:256], in_=py1[:])
                write_y(y01_own, b, evy, D1)
                if b in ag_at_pre:
                    ags01[ag_at_pre[b]]()
                if b >= PF_B:
                    emit_for(gst1, Y01, b - PF_B, WPF)

            # ================= aggregation layer (L1/L2) =====================
            def agg_layer(Ytab, wr_tiles, bias_t, h_src, h_dst, wl_next, y_next,
                          d_next, ags_next, gst, pf_gst=None, pf_tab=None):
                gtiles = gst["g"]

                for b in range(NBLK):
                    emit_for(gst, Ytab, b, WCH)
                    ohb = load_oh(b)
                    ps0 = psp.tile([128, 128], FP32, name="ps0", tag="ps0")
                    ps1 = psp.tile([128, 128], FP32, name="ps1", tag="ps1")
                    if h_src is None:
                        x0b = xsp.tile([128, 128], BF16, name="x0b2", tag="xd0")
                        nc.sync.dma_start(out=x0b[:], in_=x0dT[:, blk_sl(b)])
                        x1b = xsp.tile([128, 128], BF16, name="x1b2", tag="xd1")
                        nc.sync.dma_start(out=x1b[:], in_=x1dT[:, blk_sl(b)])
                        nc.tensor.matmul(ps0[:], lhsT=wr0t[:], rhs=x0b[:], start=True, stop=False)
                        nc.tensor.matmul(ps1[:], lhsT=wr1t[:], rhs=x1b[:], start=True, stop=False)
                    else:
                        hd0 = evp.tile([128, 128], BF16, name="hd0", tag="hd0")
                        nc.vector.tensor_tensor(out=hd0[:], in0=h_src[0][:, blk_sl(b)],
                                                in1=degr[:, blk_sl(b)], op=AL.mult)
                        hd1 = evp.tile([128, 128], BF16, name="hd1", tag="hd1")
                        nc.vector.tensor_tensor(out=hd1[:], in0=h_src[1][:, blk_sl(b)],
                                                in1=degr[:, blk_sl(b)], op=AL.mult)
                        nc.tensor.matmul(ps0[:], lhsT=wr_tiles[0][:, 0:128], rhs=hd0[:], start=True, stop=False)
                        nc.tensor.matmul(ps0[:], lhsT=wr_tiles[1][:, 0:128], rhs=hd1[:], start=False, stop=False)
                        nc.tensor.matmul(ps1[:], lhsT=wr_tiles[0][:, 128:256], rhs=hd0[:], start=True, stop=False)
                        nc.tensor.matmul(ps1[:], lhsT=wr_tiles[1][:, 128:256], rhs=hd1[:], start=False, stop=False)
                    nc.tensor.matmul(ps0[:], lhsT=bias_t[0:1, 0:128], rhs=degr[0:1, blk_sl(b)],
                                     start=False, stop=False)
                    nc.tensor.matmul(ps1[:], lhsT=bias_t[0:1, 128:256], rhs=degr[0:1, blk_sl(b)],
                                     start=False, stop=False)
                    for grp, last in pair_groups(b):
                        h, (ci, slot, paircol) = grp[0]
                        g = gtiles[ci]
                        oht, start = ohb[h]
                        j = paircol - start
                        if len(grp) == 2:
                            nc.tensor.matmul(ps0[:], lhsT=g[:, slot:slot + 2, 0:128],
                                             rhs=oht[:, j:j + 2, :], start=False, stop=last,
                                             perf_mode=mybir.MatmulPerfMode.DoubleRow)
                            nc.tensor.matmul(ps1[:], lhsT=g[:, slot:slot + 2, 128:256],
                                             rhs=oht[:, j:j + 2, :], start=False, stop=last,
                                             perf_mode=mybir.MatmulPerfMode.DoubleRow)
                        else:
                            nc.tensor.matmul(ps0[:], lhsT=g[:, slot, 0:128], rhs=oht[:, j, :],
                                             start=False, stop=last)
                            nc.tensor.matmul(ps1[:], lhsT=g[:, slot, 128:256], rhs=oht[:, j, :],
                                             start=False, stop=last)
                    # epilogue: h = relu(ps) * invd  (relu commutes with the
                    # positive per-column scale)
                    rt0 = evp.tile([128, 128], BF16, name="rt0", tag="rt0")
                    nc.scalar.activation(rt0[:], ps0[:], AF.Relu)
                    nc.vector.tensor_tensor(out=h_dst[0][:, blk_sl(b)], in0=rt0[:],
                                            in1=invr[:, blk_sl(b)], op=AL.mult)
                    rt1 = evp.tile([128, 128], BF16, name="rt1", tag="rt1")
                    nc.scalar.activation(rt1[:], ps1[:], AF.Relu)
                    nc.vector.tensor_tensor(out=h_dst[1][:, blk_sl(b)], in0=rt1[:],
                                            in1=invr[:, blk_sl(b)], op=AL.mult)
                    pyn = psyp.tile([128, d_next], FP32, name="pyn", tag="py",
                                    padded_shape=[128, 256])
                    nc.tensor.matmul(pyn[:], lhsT=h_dst[0][:, blk_sl(b)], rhs=wl_next[0][:],
                                     start=True, stop=False)
                    nc.tensor.matmul(pyn[:], lhsT=h_dst[1][:, blk_sl(b)], rhs=wl_next[1][:],
                                     start=False, stop=True)
                    evn = evp.tile([128, d_next], F8, name="evn", tag="evy",
                                   padded_shape=[128, 512])
                    nc.vector.tensor_copy(out=evn[:], in_=pyn[:])
                    write_y(y_next, b, evn, d_next)
                    if b in ag_at:
                        for hh in ag_at[b]:
                            ags_next[hh]()
                    if pf_gst is not None and b >= PF_B:
                        emit_for(pf_gst, pf_tab, b - PF_B, WPF)

            agg_layer(Y01, None, b01t, None, hT, wlmt, ym_own, DM,
                      make_ags(ym_own, Ym), gst1, gst2, Ym)
            agg_layer(Ym, wrmt, bmt, hT, h2T, wlot, yo_own, DO,
                      make_ags(yo_own, Yo), gst2, gst3, Yo)

            # ================= L3: out[node, 64] ============================
            gtiles3 = gst3["g"]

            for b in range(NBLK):
                emit_for(gst3, Yo, b, WCH)
                ohb = load_oh(b)
                ps3 = psp.tile([128, DO], FP32, name="ps3", tag="ps0",
                               padded_shape=[128, 128])
                hd0 = evp.tile([128, 128], BF16, name="hd20", tag="hd0")
                nc.vector.tensor_tensor(out=hd0[:], in0=h2T[0][:, blk_sl(b)],
                                        in1=degr[:, blk_sl(b)], op=AL.mult)
                hd1 = evp.tile([128, 128], BF16, name="hd21", tag="hd1")
                nc.vector.tensor_tensor(out=hd1[:], in0=h2T[1][:, blk_sl(b)],
                                        in1=degr[:, blk_sl(b)], op=AL.mult)
                nc.tensor.matmul(ps3[:], lhsT=hd0[:], rhs=wrot[0][:],
                                 start=True, stop=False)
                nc.tensor.matmul(ps3[:], lhsT=hd1[:], rhs=wrot[1][:],
                                 start=False, stop=False)
                nc.tensor.matmul(ps3[:], lhsT=degr[0:1, blk_sl(b)], rhs=bot[0:1, :],
                                 start=False, stop=False)
                for grp, last in pair_groups(b):
                    h, (ci, slot, paircol) = grp[0]
                    g3 = gtiles3[ci]
                    oht, start = ohb[h]
                    j = paircol - start
                    if len(grp) == 2:
                        nc.tensor.matmul(ps3[:], lhsT=oht[:, j:j + 2, :],
                                         rhs=g3[:, slot:slot + 2, 0:64],
                                         start=False, stop=last,
                                         perf_mode=mybir.MatmulPerfMode.DoubleRow)
                    else:
                        nc.tensor.matmul(ps3[:], lhsT=oht[:, j, :], rhs=g3[:, slot, 0:64],
                                         start=False, stop=last)
                osb = evp.tile([128, DO], FP32, name="osb", tag="osb")
                nc.scalar.activation(osb[:], ps3[:], AF.Copy,
                                     scale=invc[:, b:b + 1])
                rows = min(128, S - b * 128)
                nc.sync.dma_start(out=outd[b * 128: b * 128 + rows, :],
                                  in_=osb[0:rows, :])

    nc.finalize()
    return nc


_CACHE = {}


def _make_inmaps(inputs, pre, calls, pair_list):
    import ml_dtypes as _ml
    BF = _ml.bfloat16
    x0 = np.asarray(inputs["x0"], np.float32)
    x1 = np.asarray(inputs["x1"], np.float32)
    deg = pre["deg"]
    bf16 = lambda a: np.ascontiguousarray(a).astype(BF)
    in_maps = []
    for c in range(NCORES):
        degc = np.maximum(deg[c], 1.0).astype(np.float32)
        invd = (1.0 / degc).astype(np.float32)
        degc_p = np.ones(SP, np.float32)
        degc_p[:S] = degc
        invd_p = np.ones(SP, np.float32)
        invd_p[:S] = invd
        idx_img, dstloc = _idx_arrays(pre, calls, pair_list, c)
        x0c = np.zeros((128, SP), np.float32)
        x0c[:, :S] = x0[c * S:(c + 1) * S, :].T
        x1c = np.zeros((128, SP), np.float32)
        x1c[:, :S] = x1[c * S:(c + 1) * S, :].T
        x0dc = x0c * degc_p[None, :]
        x1dc = x1c * degc_p[None, :]
        in_maps.append({
            "x0T": bf16(x0c), "x1T": bf16(x1c),
            "x0dT": bf16(x0dc), "x1dT": bf16(x1dc),
            "wl0": bf16(inputs["Wl0"]), "wr0": bf16(inputs["Wr0"]),
            "wl1": bf16(inputs["Wl1"]), "wr1": bf16(inputs["Wr1"]),
            "wlm": bf16(inputs["Wlm"]), "wrm": bf16(inputs["Wrm"]),
            "wlo": bf16(inputs["Wlo"]), "wro": bf16(inputs["Wro"]),
            "b01": bf16(np.concatenate([np.asarray(inputs["b0"], np.float32),
                                        np.asarray(inputs["b1"], np.float32)])[None, :]),
            "bm": bf16(np.asarray(inputs["bm"], np.float32)[None, :]),
            "bo": bf16(np.asarray(inputs["bo"], np.float32)[None, :]),
            "idx": idx_img, "dstl": dstloc,
            "invr": bf16(np.broadcast_to(invd_p[None, :], (128, SP))),
            "degr": bf16(np.broadcast_to(degc_p[None, :], (128, SP))),
            "invc": np.ascontiguousarray(invd_p[:NBLK * 128].reshape(NBLK, 128).T,
                                         np.float32),
        })
    return in_maps


def _get_program(edge_index):
    if "prog" in _CACHE:
        return _CACHE["prog"]
    pre = _preprocess(edge_index)
    calls, block_pairs, oh_rng, pair_list, mth = _build_callplan(pre)
    nc = _build_bass(pre, calls, block_pairs, oh_rng, len(pair_list), mth)
    _CACHE["prog"] = (nc, pre, calls, pair_list)
    return _CACHE["prog"]


LAST_EXEC_NS = None


def kernel(**inputs):
    global LAST_EXEC_NS
    _install_hooks()
    from concourse.bass_utils import run_bass_kernel_spmd

    nc, pre, calls, pair_list = _get_program(inputs["edge_index"])
    in_maps = _make_inmaps(inputs, pre, calls, pair_list)
    trace = os.environ.get("KERNEL_TRACE", "0") == "1"
    res = run_bass_kernel_spmd(nc, in_maps, list(range(NCORES)), trace=trace)
    LAST_EXEC_NS = res.exec_time_ns
    return np.concatenate([np.asarray(res.results[c]["out"]) for c in range(NCORES)], axis=0)
